# revision 51
# baseline (speedup 1.0000x reference)
"""DCRNN cell (diffusion conv GRU step, K=3) on 8 trn2 NeuronCores.

Sharding: nodes are assigned to 8 cores x SB blocks of 128 slots by a greedy
2-D balanced bin packing (in-degree and out-degree per bin).  Each core owns
the edges whose destination falls in its node range (per direction), does
gather (indirect DMA, 4 SWDGE queues round-robin) + one-hot-selector matmul
scatter into PSUM for both diffusion hops, with one AllGather halo exchange
of the scaled hop-1 results between hops.  Gates/head are dense matmuls on
the owned slice.

Self-loop edges (row==col from the explicit loop set) are pulled out of the
edge lists and applied as local per-node terms, which drops the per-block
chunk count.  The source-node table is split at SPLIT (not NS/2) so the two
int16 index halves pack chunks tighter (5+4 instead of 6+6 per block).

Since H0 = 0 in the reference, only the first IN_CH rows of the gate weights
matter and the R gate has no effect on the output; this kernel exploits both.
"""

import os
import sys

for _p in ("/opt/pypackages", "/opt/trn_rl_repo"):
    if _p not in sys.path:
        sys.path.insert(0, _p)

from contextlib import ExitStack

import numpy as np

import concourse.bass as bass
import concourse.mybir as mybir
import concourse.tile as tile
from concourse import bacc
from concourse.bass import AP
from concourse.library_config import mlp as mlp_library
from concourse.masks import make_identity

F16 = mybir.dt.float16
F32 = mybir.dt.float32
I16 = mybir.dt.int16
I32 = mybir.dt.int32

N_CORES = 8
P = 128  # partitions / block size
WG = 4  # dst blocks per gather window


def _ceil_div(a, b):
    return -(-a // b)


# ----------------------------------------------------------------------------
# Host-side prep: permutation, edge bucketing, padded layouts (index work only)
# ----------------------------------------------------------------------------


class HostPlan:
    pass


def host_prep(x, edge_index, edge_weight):
    n, IN = x.shape
    row = edge_index[0].astype(np.int64)
    col = edge_index[1].astype(np.int64)
    w = edge_weight.astype(np.float32)
    E = row.shape[0]

    SB = _ceil_div(n, N_CORES * P)  # blocks per core
    NS = N_CORES * SB * P  # total node slots
    SBB = SB * P  # slots per core

    # deg/xtab group size: nodes per partition-row per pipeline step
    nrow = NS // P
    G = max(g for g in range(1, 29) if nrow % g == 0)
    NBG = nrow // G  # pipeline groups

    # split point of the source table (int16 index range per half)
    if os.environ.get("KERNEL_FORCE_SPLIT") and NS > G * P:
        SPLIT = (NS // (2 * G * P)) * G * P  # debug: exercise split path
    elif NS <= 32768:
        SPLIT = NS
    else:
        tgt = int(round(0.5714 * NS / (G * P))) * G * P
        SPLIT = min(32768 // (G * P) * (G * P), max(NS - 32768, tgt))
    assert SPLIT <= 32768 and NS - SPLIT <= 32768 and SPLIT % (G * P) == 0

    # --- balanced assignment of nodes to (core, block) bins ---
    din = np.bincount(col, minlength=n).astype(np.float64)
    dout = np.bincount(row, minlength=n).astype(np.float64)
    nbins = N_CORES * SB
    order = np.argsort(-(din + dout), kind="stable")
    in_load = np.zeros(nbins)
    out_load = np.zeros(nbins)
    cap = np.full(nbins, P, np.int64)
    binof = np.empty(n, np.int64)
    for nd in order:
        score = (in_load + din[nd]) ** 2 + (out_load + dout[nd]) ** 2
        score[cap == 0] = np.inf
        b = int(np.argmin(score))
        binof[nd] = b
        in_load[b] += din[nd]
        out_load[b] += dout[nd]
        cap[b] -= 1
    node2g = np.empty(n, np.int64)
    o = np.argsort(binof, kind="stable")
    rank = np.arange(n) - np.searchsorted(binof[o], binof[o])
    node2g[o] = binof[o] * P + rank

    xg = np.zeros((NS, IN), np.float32)
    xg[node2g] = x

    # --- padded per-node weight lists (for degree computation; incl. loops) ---
    def wpad(keys_g):
        o = np.argsort(keys_g, kind="stable")
        ks = keys_g[o]
        ws = w[o]
        starts = np.searchsorted(ks, np.arange(NS))
        r = np.arange(E) - starts[ks]
        cdeg = max(8, int(_ceil_div(int(r.max()) + 1, 4) * 4))
        W = np.zeros((NS, cdeg), np.float16)
        W[ks, r] = ws.astype(np.float16)
        c = np.bincount(ks, minlength=NS)
        W[c == 0, 0] = 1.0  # pad/isolated nodes: deg := 1 (never used)
        return W, cdeg

    wpo, cdeg_o = wpad(node2g[row])
    wpi, cdeg_i = wpad(node2g[col])
    CDEG = max(cdeg_o, cdeg_i)
    if cdeg_o < CDEG:
        wpo = np.pad(wpo, ((0, 0), (0, CDEG - cdeg_o)))
    if cdeg_i < CDEG:
        wpi = np.pad(wpi, ((0, 0), (0, CDEG - cdeg_i)))

    # --- per-direction edge bucketing ---
    # Pull out self-loop edges and apply them as local per-node terms instead
    # of gathered edges.  selfc[slot] counts the excluded edges per node (the
    # device multiplies the local term by it), so any multiplicity of
    # self-edges — or none — stays exact.
    selfm = row == col
    selfc_n = np.bincount(row[selfm], minlength=n).astype(np.float32)
    selfc = np.zeros(NS, np.float32)
    selfc[node2g] = selfc_n
    nonself = ~selfm
    wins = [range(s, min(s + WG, SB)) for s in range(0, SB, WG)]

    def make_dir(src_g, dst_g):
        Ed = src_g.shape[0]
        owner = dst_g // SBB
        blk = (dst_g % SBB) // P
        dslot = dst_g % P
        half = (src_g >= SPLIT).astype(np.int64)
        idxv = (src_g - half * SPLIT).astype(np.int64)
        o = np.lexsort((half, blk, owner))
        owner_s, blk_s, half_s = owner[o], blk[o], half[o]
        idx_s, dslot_s = idxv[o], dslot[o]
        # chunk capacity per (block, half): max over cores
        counts = np.zeros((N_CORES, SB, 2), np.int64)
        np.add.at(counts, (owner_s, blk_s, half_s), 1)
        C = _ceil_div(counts, P).max(axis=0)  # [SB, 2]
        # flat chunk layout: for win: for half: for blk in win
        start_chunk = np.zeros((SB, 2), np.int64)
        ct = 0
        for wi in wins:
            for h in (0, 1):
                for b in wi:
                    start_chunk[b, h] = ct
                    ct += C[b, h]
        NCH = ct
        EF = NCH * P
        # scatter edges into flat arrays
        gk = (owner_s * SB + blk_s) * 2 + half_s
        gstart = np.searchsorted(gk, np.arange(N_CORES * SB * 2))
        r = np.arange(Ed) - gstart[gk]
        posf = start_chunk[blk_s, half_s] * P + r
        idx_flat = np.zeros((N_CORES, EF), np.int16)
        d_flat = np.full((N_CORES, EF), -1.0, np.float16)
        idx_flat[owner_s, posf] = idx_s.astype(np.int16)
        d_flat[owner_s, posf] = dslot_s.astype(np.float16)
        # device layouts
        idx_t = np.ascontiguousarray(
            np.tile(idx_flat.reshape(N_CORES, EF // 16, 16).transpose(0, 2, 1), (1, 8, 1))
        )  # [N_CORES, 128, EF//16]
        d_t = np.ascontiguousarray(d_flat.reshape(N_CORES, EF // P, P).transpose(0, 2, 1))
        d = HostPlan()
        d.C = C
        d.start_chunk = start_chunk
        d.NCH = NCH
        d.EF = EF
        d.idx_t = idx_t
        d.d_t = d_t
        return d

    fwd = make_dir(node2g[row[nonself]], node2g[col[nonself]])
    rev = make_dir(node2g[col[nonself]], node2g[row[nonself]])

    pl = HostPlan()
    pl.n, pl.IN, pl.SB, pl.NS, pl.SBB, pl.CDEG = n, IN, SB, NS, SBB, CDEG
    pl.SPLIT, pl.G, pl.NBG = SPLIT, G, NBG
    pl.wins = wins
    pl.node2g = node2g
    pl.xg = xg
    pl.selfc = selfc
    pl.wpo, pl.wpi = wpo, wpi
    pl.fwd, pl.rev = fwd, rev
    return pl


# ----------------------------------------------------------------------------
# Device program
# ----------------------------------------------------------------------------


def build_program(pl, OUT, OSZ, gq_plan=None):
    """OUT: gate output channels (128); OSZ: final head size (12).

    gq_plan: optional list mapping gather emission index -> SWDGE queue.
    The Tile scheduler assigns DMASW sem lanes round-robin in *scheduled*
    order and each lane is locked to one queue, so queue numbers must equal
    scheduled_position %% 4.  build() runs twice: pass 1 with all-queue-0
    discovers the schedule, pass 2 bakes the matching queue plan.
    """
    IN, SB, NS, SBB, SPLIT, CDEG = pl.IN, pl.SB, pl.NS, pl.SBB, pl.SPLIT, pl.CDEG
    G, NBG = pl.G, pl.NBG
    NBG0 = SPLIT // (G * P)  # groups in half 0
    NBG1 = NBG - NBG0

    nc = bacc.Bacc(
        "TRN2", target_bir_lowering=False, debug=False, num_devices=N_CORES,
        enable_asserts=False, num_swdge_queues=4,
    )

    # ---- I/O ----
    xg_d = nc.dram_tensor("xg", [NS, IN], F32, kind="ExternalInput").ap()
    wpo_d = nc.dram_tensor("wpo", [NS, CDEG], F16, kind="ExternalInput").ap()
    wpi_d = nc.dram_tensor("wpi", [NS, CDEG], F16, kind="ExternalInput").ap()
    xm_d = nc.dram_tensor("xm", [SBB, IN], F32, kind="ExternalInput").ap()
    wpom_d = nc.dram_tensor("wpom", [SBB, CDEG], F16, kind="ExternalInput").ap()
    wpim_d = nc.dram_tensor("wpim", [SBB, CDEG], F16, kind="ExternalInput").ap()
    fidx_d = nc.dram_tensor("fidx", [P, pl.fwd.EF // 16], I16, kind="ExternalInput").ap()
    fd_d = nc.dram_tensor("fd", [P, pl.fwd.EF // P], F16, kind="ExternalInput").ap()
    ridx_d = nc.dram_tensor("ridx", [P, pl.rev.EF // 16], I16, kind="ExternalInput").ap()
    rd_d = nc.dram_tensor("rd", [P, pl.rev.EF // P], F16, kind="ExternalInput").ap()
    wz_d = nc.dram_tensor("wz", [2, 3, IN, OUT], F32, kind="ExternalInput").ap()
    wh_d = nc.dram_tensor("wh", [2, 3, IN, OUT], F32, kind="ExternalInput").ap()
    bz_d = nc.dram_tensor("bzc", [OUT, 1], F32, kind="ExternalInput").ap()
    bh_d = nc.dram_tensor("bhc", [OUT, 1], F32, kind="ExternalInput").ap()
    wl_d = nc.dram_tensor("wl", [OUT, OSZ], F32, kind="ExternalInput").ap()
    blr_d = nc.dram_tensor("blr", [P, OSZ], F32, kind="ExternalInput").ap()
    sfc_d = nc.dram_tensor("sfc", [P, SB], F32, kind="ExternalInput").ap()
    out_d = nc.dram_tensor("out", [SBB, OSZ], F32, kind="ExternalOutput").ap()

    xtab0 = nc.dram_tensor("xtab0", [SPLIT, 2 * IN], F16, kind="Internal").ap()
    xtab1 = (
        nc.dram_tensor("xtab1", [NS - SPLIT, 2 * IN], F16, kind="Internal").ap()
        if NBG1 > 0 else None
    )
    t1slice = nc.dram_tensor("t1slice", [SBB, 2 * IN], F16, kind="Internal").ap()
    t1tab = nc.dram_tensor(
        "t1tab", [NS, 2 * IN], F16, kind="Internal", addr_space="Shared"
    ).ap()

    with tile.TileContext(nc) as tc:
        sbuf = lambda nm, sh, dt: nc.alloc_sbuf_tensor(nm, sh, dt).ap()

        nc.gpsimd.load_library(mlp_library)

        # ---- static SBUF ----
        iota_i = sbuf("iota_i", [P, P], I32)
        iota16 = sbuf("iota16", [P, P], F16)
        ident = sbuf("ident", [P, P], F32)
        ident16 = sbuf("ident16", [P, P], F16)
        fidx_s = sbuf("fidx_s", [P, pl.fwd.EF // 16], I16)
        fd_s = sbuf("fd_s", [P, pl.fwd.EF // P], F16)
        ridx_s = sbuf("ridx_s", [P, pl.rev.EF // 16], I16)
        rd_s = sbuf("rd_s", [P, pl.rev.EF // P], F16)
        recm_o = sbuf("recm_o", [P, SB], F32)  # my recip deg, block-col packed
        recm_i = sbuf("recm_i", [P, SB], F32)
        sfc_s = sbuf("sfc_s", [P, SB], F32)  # self-edge count per slot
        X_sb = sbuf("X_sb", [P, SB * IN], F32)  # my x, block-col packed
        # local-term buffers, added into PSUM via identity "ghost" matmuls:
        # hold XR = x*recm*selfc during hop 1, then are overwritten with
        # B2h = selfc*t1loc - x/2 for hop 2 (WAR dep orders the overwrite
        # after the last hop-1 read)
        XR_o = sbuf("XR_o", [P, SB * IN], F16)
        XR_i = sbuf("XR_i", [P, SB * IN], F16)
        B2_o = XR_o
        B2_i = XR_i
        To1_sb = sbuf("To1_sb", [P, SB * IN], F16)
        Ti1_sb = sbuf("Ti1_sb", [P, SB * IN], F16)
        To2_sb = sbuf("To2_sb", [P, SB * IN], F16)
        Ti2_sb = sbuf("Ti2_sb", [P, SB * IN], F16)
        T1o_st = sbuf("T1o_st", [P, SB * IN], F16)  # staged t1 rows (per dir)
        T1i_st = sbuf("T1i_st", [P, SB * IN], F16)
        bz_s = sbuf("bz_s", [OUT, 1], F32)
        bh_s = sbuf("bh_s", [OUT, 1], F32)
        wl_s = sbuf("wl_s", [OUT, OSZ], F16)
        blr_s = sbuf("blr_s", [P, OSZ], F32)
        BZ = [sbuf(f"BZ{i}", [IN, OUT], F16) for i in range(5)]
        BH = [sbuf(f"BH{i}", [IN, OUT], F16) for i in range(5)]
        rec_all = {nm: sbuf(f"recg_{nm}", [P, G * NBG], F32) for nm in ("o", "i")}

        nc.gpsimd.iota(iota_i, [[1, P]], channel_multiplier=0)
        nc.vector.tensor_copy(iota16, iota_i)
        make_identity(nc, ident)
        nc.vector.tensor_copy(ident16, ident)
        nc.sync.dma_start(fidx_s, fidx_d)
        nc.sync.dma_start(fd_s, fd_d)
        nc.sync.dma_start(ridx_s, ridx_d)
        nc.sync.dma_start(rd_s, rd_d)
        nc.sync.dma_start(bz_s, bz_d)
        nc.sync.dma_start(bh_s, bh_d)
        nc.sync.dma_start(blr_s, blr_d)
        nc.sync.dma_start(sfc_s, sfc_d)

        # X_sb: partition p = node slot 128b+p of my range
        nc.sync.dma_start(
            AP(X_sb.tensor, 0, [[SB * IN, P], [IN, SB], [1, IN]]),
            AP(xm_d.tensor, 0, [[IN, P], [P * IN, SB], [1, IN]]),
        )

        # all pools stay open together so the prefix, hop, and gate phases
        # never serialize on SBUF space reuse (WAR on pool buffers)
        stack = ExitStack()
        dsb = stack.enter_context(tc.tile_pool(name="degs", bufs=2))
        one = stack.enter_context(tc.tile_pool(name="oneshot", bufs=1))
        gw = stack.enter_context(tc.tile_pool(name="gwin", bufs=2))
        sp = stack.enter_context(tc.tile_pool(name="strip", bufs=3))
        pp = stack.enter_context(tc.tile_pool(name="pprop", bufs=4, space="PSUM"))
        gs = stack.enter_context(tc.tile_pool(name="gts", bufs=2))
        gp = stack.enter_context(tc.tile_pool(name="gtp", bufs=2, space="PSUM"))
        gp2 = stack.enter_context(tc.tile_pool(name="gtp2", bufs=2, space="PSUM"))

        # ---- phase 1: degrees + reciprocals + xtab build, pipelined ----
        # xtab row g = [x[g]/deg_out[g] | x[g]/deg_in[g]] fp16.  Emission
        # order front-loads the critical path: xtab half 0 first (gates the
        # first gathers), then recm/XR (hop-1 posts), then half 1, then the
        # gate weights (needed last).
        def emit_group(j):
            rb4 = {}
            for nm, wsrc in (("o", wpo_d), ("i", wpi_d)):
                rec = rec_all[nm]
                wt = dsb.tile([P, G * CDEG], F16, tag=f"wt{nm}")
                nc.sync.dma_start(
                    wt[:],
                    AP(wsrc.tensor, j * G * P * CDEG,
                       [[G * CDEG, P], [1, G * CDEG]]),
                )
                dg = dsb.tile([P, G], F32, tag=f"dg{nm}")
                nc.vector.tensor_reduce(
                    dg[:],
                    wt[:].rearrange("p (j c) -> p j c", c=CDEG),
                    axis=mybir.AxisListType.X, op=mybir.AluOpType.add,
                )
                nc.vector.reciprocal(rec[:, G * j : G * j + G], dg[:])
                rb4[nm] = rec[:, G * j : G * j + G]
            xt = dsb.tile([P, G * IN], F32, tag="xin")
            nc.scalar.dma_start(
                xt[:],
                AP(xg_d.tensor, j * G * P * IN, [[G * IN, P], [1, G * IN]]),
            )
            ot = dsb.tile([P, G * 2 * IN], F16, tag="xout")
            xt3 = xt[:].rearrange("p (j f) -> p j f", f=IN)
            ot3 = ot[:].rearrange("p (j f) -> p j f", f=2 * IN)
            # before the first gathers the GpSimd engine is idle: let it do
            # half the xtab multiplies for the gather-gating half 0
            eng = nc.gpsimd if (j < NBG0 and j % 2 == 1) else nc.vector
            for hx, nm in ((0, "o"), (1, "i")):
                rb = rb4[nm].unsqueeze(2).to_broadcast([P, G, IN])
                eng.tensor_tensor(
                    ot3[:, :, hx * IN : (hx + 1) * IN], xt3, rb,
                    op=mybir.AluOpType.mult,
                )
            dstt, jj = (xtab0, j) if j < NBG0 else (xtab1, j - NBG0)
            nc.sync.dma_start(
                AP(dstt.tensor, jj * G * P * 2 * IN,
                   [[G * 2 * IN, P], [1, G * 2 * IN]]),
                ot[:],
            )

        for j in range(NBG0):
            emit_group(j)

        # my recips, block-column packing (one DMA + reduce per direction)
        for nm, wsrc, dst in (("o", wpom_d, recm_o), ("i", wpim_d, recm_i)):
            wt = one.tile([P, SB * CDEG], F16, tag=f"wtm{nm}")
            nc.scalar.dma_start(
                wt[:],
                AP(wsrc.tensor, 0, [[CDEG, P], [P * CDEG, SB], [1, CDEG]]),
            )
            dg = one.tile([P, SB], F32, tag=f"dgm{nm}")
            nc.vector.tensor_reduce(
                dg[:],
                wt[:].rearrange("p (j c) -> p j c", c=CDEG),
                axis=mybir.AxisListType.X, op=mybir.AluOpType.add,
            )
            nc.vector.reciprocal(dst, dg[:])

        # self-loop terms: XR = X * recm * selfc (per-direction)
        X3 = X_sb[:].rearrange("p (b f) -> p b f", f=IN)
        rs_o = sbuf("rs_o", [P, SB], F32)
        rs_i = sbuf("rs_i", [P, SB], F32)
        for recm, rs, XR in ((recm_o, rs_o, XR_o), (recm_i, rs_i, XR_i)):
            nc.vector.tensor_tensor(rs, recm, sfc_s, op=mybir.AluOpType.mult)
            nc.vector.tensor_tensor(
                XR[:].rearrange("p (b f) -> p b f", f=IN),
                X3,
                rs[:].unsqueeze(2).to_broadcast([P, SB, IN]),
                op=mybir.AluOpType.mult,
            )

        for j in range(NBG0, NBG):
            emit_group(j)

        # ---- weights prep ----
        for (src, dst) in ((wz_d, BZ), (wh_d, BH)):
            t0 = one.tile([IN, OUT], F32, tag="w0")
            t1 = one.tile([IN, OUT], F32, tag="w1")
            nc.scalar.dma_start(t0, src[0, 0])
            nc.scalar.dma_start(t1, src[1, 0])
            nc.vector.tensor_tensor(dst[0], t0, t1, op=mybir.AluOpType.add)
            for k, (di, ki) in enumerate(((0, 1), (1, 1), (0, 2), (1, 2))):
                tk = one.tile([IN, OUT], F32, tag=f"wk{k}")
                nc.scalar.dma_start(tk, src[di, ki])
                nc.vector.tensor_copy(dst[1 + k], tk)
        twl = one.tile([OUT, OSZ], F32, tag="wl")
        nc.scalar.dma_start(twl, wl_d)
        nc.vector.tensor_copy(wl_s, twl)

        # ---- gather-scatter hop machinery ----
        gq = [0]  # gather emission counter
        gather_names = []  # emission-order instruction names

        def run_hops(jobs, win_cb=None):
            """jobs: list of (dirp, idx_s, d_s, (tab_lo, tab_hi), coloff, loc,
            post).  loc[:, b*IN:] is added into the block's PSUM via an
            identity ghost matmul; post(b, psum_ap) consumes the result.
            win_cb(blocks) is emitted after each window's jobs so downstream
            per-block work interleaves with the next window's gathers."""
            if True:
                for wi in pl.wins:
                    blocks = list(wi)
                    for jid, (dirp, idx_s, d_s, tabs, coloff, loc, post) in enumerate(jobs):
                        C = dirp.C
                        nch_h = [sum(int(C[b, h]) for b in blocks) for h in (0, 1)]
                        nw = nch_h[0] + nch_h[1]
                        if nw == 0:
                            continue
                        wbuf = gw.tile([P, nw, 2 * IN], F16, tag=f"wb{jid}")
                        ct0 = int(dirp.start_chunk[blocks[0], 0])
                        for h in (0, 1):
                            ni = nch_h[h] * P
                            if ni == 0:
                                continue
                            co = 0 if h == 0 else nch_h[0]
                            qn = gq_plan[gq[0]] if gq_plan else 0
                            gi = nc.gpsimd.dma_gather(
                                wbuf[:, co : co + nch_h[h], :],
                                tabs[h],
                                idx_s[:, (ct0 + co) * 8 : (ct0 + co) * 8 + ni // 16],
                                ni, ni, 2 * IN,
                                single_packet=False,
                                queue_num=qn,
                            )
                            gather_names.append(gi.ins.name)
                            gq[0] += 1
                        # one-hot selector strips for the whole window
                        st = sp.tile([P, nw, P], F16, tag="st")
                        nc.vector.tensor_tensor(
                            st[:],
                            iota16.unsqueeze(1).to_broadcast([P, nw, P]),
                            d_s[:, ct0 : ct0 + nw].unsqueeze(2).to_broadcast([P, nw, P]),
                            op=mybir.AluOpType.is_equal,
                        )
                        for b in blocks:
                            tc_chunks = []
                            for h in (0, 1):
                                s = int(dirp.start_chunk[b, h])
                                for k in range(int(C[b, h])):
                                    tc_chunks.append(s + k)
                            ps = pp.tile([P, IN], F32, tag="ps")
                            for i, ct in enumerate(tc_chunks):
                                lw = ct - ct0  # local chunk within window buffer
                                nc.tensor.matmul(
                                    ps[:],
                                    lhsT=st[:, lw, :],
                                    rhs=wbuf[:, lw, coloff : coloff + IN],
                                    start=(i == 0),
                                    stop=False,
                                )
                            # local term: ps += I.T @ loc_b
                            nc.tensor.matmul(
                                ps[:],
                                lhsT=ident16,
                                rhs=loc[:, b * IN : (b + 1) * IN],
                                start=(len(tc_chunks) == 0),
                                stop=True,
                            )
                            post(b, ps)
                    if win_cb is not None:
                        win_cb(blocks)

        # ---- phase 3: hop 1 (+ t1 staging) ----
        # ghost matmul already added XR into ps; posts run on the Scalar engine
        def post_hop1(To_sb, recm, T1st):
            def post(b, ps):
                nc.scalar.copy(To_sb[:, b * IN : (b + 1) * IN], ps[:])
                nc.scalar.activation(
                    T1st[:, b * IN : (b + 1) * IN], ps[:],
                    mybir.ActivationFunctionType.Copy,
                    scale=recm[:, b : b + 1],
                )
            return post

        def stage_t1(blocks):
            b0, nb = blocks[0], len(blocks)
            for hx, T1st in ((0, T1o_st), (1, T1i_st)):
                nc.sync.dma_start(
                    AP(t1slice.tensor, b0 * P * 2 * IN + hx * IN,
                       [[2 * IN, P], [P * 2 * IN, nb], [1, IN]]),
                    T1st[:, b0 * IN : (b0 + nb) * IN].rearrange(
                        "p (b f) -> p b f", f=IN),
                )

        xtabs = (xtab0, xtab1 if xtab1 is not None else xtab0)
        run_hops([
            (pl.fwd, fidx_s, fd_s, xtabs, 0, XR_o, post_hop1(To1_sb, recm_o, T1o_st)),
            (pl.rev, ridx_s, rd_s, xtabs, IN, XR_i, post_hop1(Ti1_sb, recm_i, T1i_st)),
        ], win_cb=stage_t1)
        if os.environ.get("KERNEL_NO_COLL"):
            # debug: skip cross-core exchange (numerically wrong on >1 core)
            for m in range(N_CORES):
                nc.sync.dma_start(
                    t1tab[m * SBB : (m + 1) * SBB], t1slice
                )
        else:
            nc.gpsimd.collective_compute(
                "AllGather",
                mybir.AluOpType.bypass,
                replica_groups=[list(range(N_CORES))],
                ins=[t1slice],
                outs=[t1tab],
            )

        # hop2 local terms for the ghost matmul: B2h = selfc * t1loc - X/2
        # (final T2 = 2*ps then gives 2*selfc*t1loc - X + 2*chunks)
        for T1st, B2 in ((T1o_st, B2_o), (T1i_st, B2_i)):
            B23 = B2[:].rearrange("p (b f) -> p b f", f=IN)
            nc.vector.tensor_tensor(
                B23,
                T1st[:].rearrange("p (b f) -> p b f", f=IN),
                sfc_s[:].unsqueeze(2).to_broadcast([P, SB, IN]),
                op=mybir.AluOpType.mult,
            )
            nc.vector.scalar_tensor_tensor(
                B23, X3, -0.5, B23,
                op0=mybir.AluOpType.mult,
                op1=mybir.AluOpType.add,
            )

        # ---- phase 4: hop 2 ----
        def post_hop2(T2_sb):
            def post(b, ps):
                nc.scalar.activation(
                    T2_sb[:, b * IN : (b + 1) * IN], ps[:],
                    mybir.ActivationFunctionType.Copy, scale=2.0,
                )
            return post

        # ---- phase 5: gates + head (emitted per hop-2 window) ----
        GB = 2  # blocks per gate group
        comps = [X_sb, To1_sb, Ti1_sb, To2_sb, Ti2_sb]

        def emit_gates(b0, nb):
            compT = []
            for ci, csb in enumerate(comps):
                pT = gp.tile([IN, GB * P], F32, tag="pT")
                if csb.tensor.dtype == F16:
                    pT16 = pT[:].bitcast(F16)
                    for jj in range(nb):
                        nc.tensor.transpose(
                            pT16[:, jj * P : (jj + 1) * P],
                            csb[:, (b0 + jj) * IN : (b0 + jj + 1) * IN],
                            ident16,
                        )
                    psrc = pT16
                else:
                    for jj in range(nb):
                        nc.tensor.transpose(
                            pT[:, jj * P : (jj + 1) * P],
                            csb[:, (b0 + jj) * IN : (b0 + jj + 1) * IN],
                            ident,
                        )
                    psrc = pT[:]
                cT = gs.tile([IN, GB * P], F16, tag=f"cT{ci}")
                nc.scalar.copy(cT[:, : nb * P], psrc[:, : nb * P])
                compT.append(cT)
            res = {}
            for nm, BW, bias, fn in (
                ("z", BZ, bz_s, mybir.ActivationFunctionType.Sigmoid),
                ("h", BH, bh_s, mybir.ActivationFunctionType.Tanh),
            ):
                pg = gp2.tile([OUT, GB * P], F32, tag="pg")
                for ci in range(5):
                    nc.tensor.matmul(
                        pg[:, : nb * P],
                        lhsT=BW[ci],
                        rhs=compT[ci][:, : nb * P],
                        start=(ci == 0),
                        stop=(ci == 4),
                    )
                act = gs.tile([OUT, GB * P], F16, tag=f"act{nm}")
                nc.scalar.activation(act[:, : nb * P], pg[:, : nb * P], fn, bias=bias)
                res[nm] = act
            omz = gs.tile([OUT, GB * P], F16, tag="omz")
            nc.scalar.activation(
                omz[:, : nb * P], res["z"][:, : nb * P],
                mybir.ActivationFunctionType.Copy, bias=1.0, scale=-1.0,
            )
            hT = gs.tile([OUT, GB * P], F16, tag="hT")
            nc.vector.tensor_tensor(
                hT[:, : nb * P], omz[:, : nb * P], res["h"][:, : nb * P],
                op=mybir.AluOpType.mult,
            )
            hR = gs.tile([OUT, GB * P], F16, tag="hR")
            nc.scalar.activation(
                hR[:, : nb * P], hT[:, : nb * P], mybir.ActivationFunctionType.Relu,
            )
            osb = gs.tile([P, GB * OSZ], F32, tag="osb")
            for jj in range(nb):
                ph = gp2.tile([P, OSZ], F32, tag="pg")
                nc.tensor.matmul(
                    ph[:], lhsT=hR[:, jj * P : (jj + 1) * P], rhs=wl_s,
                    start=True, stop=True,
                )
                nc.vector.tensor_tensor(
                    osb[:, jj * OSZ : (jj + 1) * OSZ], ph[:], blr_s,
                    op=mybir.AluOpType.add,
                )
            nc.scalar.dma_start(
                AP(out_d.tensor, b0 * P * OSZ,
                   [[OSZ, P], [P * OSZ, nb], [1, OSZ]]),
                osb[:].rearrange("p (j s) -> p j s", s=OSZ)[:, :nb, :],
            )

        def gates_cb(blocks):
            b0, nw = blocks[0], len(blocks)
            for g0 in range(b0, b0 + nw, GB):
                emit_gates(g0, min(GB, b0 + nw - g0))

        t1tabs = (t1tab, t1tab[SPLIT:] if SPLIT < NS else t1tab)
        run_hops([
            (pl.fwd, fidx_s, fd_s, t1tabs, 0, B2_o, post_hop2(To2_sb)),
            (pl.rev, ridx_s, rd_s, t1tabs, IN, B2_i, post_hop2(Ti2_sb)),
        ], win_cb=gates_cb)

        stack.close()

    nc.compile()
    nc._gather_names = gather_names
    return nc


def _swdge_sched_order(nc):
    """Names of Pool-engine SWDGE DMA instructions in scheduled order."""
    import concourse.mybir as mb

    names = []
    for bb in nc.m.functions[0].blocks:
        for inst in bb.instructions:
            if isinstance(inst, mb.InstDMAGatherAnt):
                names.append(inst.name)
    return names


def build_program_queued(pl, OUT, OSZ):
    """Two-pass build: discover the scheduled SWDGE order, then assign
    queue = scheduled_position %% 4 so DMASW sem lanes stay single-queue."""
    nc = build_program(pl, OUT, OSZ)
    sched = _swdge_sched_order(nc)
    emit_idx = {nm: i for i, nm in enumerate(nc._gather_names)}
    if sched and len(sched) == len(emit_idx):
        plan = [0] * len(sched)
        for pos, nm in enumerate(sched):
            plan[emit_idx[nm]] = pos % 4
        nc2 = build_program(pl, OUT, OSZ, gq_plan=plan)
        # verify lane/queue consistency under the (identical) schedule
        sched2 = _swdge_sched_order(nc2)
        emit2 = {nm: i for i, nm in enumerate(nc2._gather_names)}
        lane_q = {}
        ok = len(sched2) == len(plan)
        if ok:
            for pos, nm in enumerate(sched2):
                lane, q = pos % 8, plan[emit2[nm]]
                if lane_q.setdefault(lane, q) != q:
                    ok = False
                    break
        if ok:
            return nc2
    print("kernel: SWDGE queue plan fell back to single-queue", file=sys.stderr)
    return nc  # fall back to single-queue (correct, slower)


# ----------------------------------------------------------------------------
# Entry
# ----------------------------------------------------------------------------


def _in_maps(pl, Wz, Wh, bz, bh, Wl, bl):
    IN, OUT = pl.IN, Wz.shape[-1]
    shared = dict(
        xg=pl.xg,
        wpo=pl.wpo,
        wpi=pl.wpi,
        wz=np.ascontiguousarray(Wz[:, :, :IN, :], np.float32),
        wh=np.ascontiguousarray(Wh[:, :, :IN, :], np.float32),
        bzc=np.ascontiguousarray(bz.reshape(OUT, 1), np.float32),
        bhc=np.ascontiguousarray(bh.reshape(OUT, 1), np.float32),
        wl=np.ascontiguousarray(Wl, np.float32),
        blr=np.ascontiguousarray(np.tile(bl.reshape(1, -1), (P, 1)), np.float32),
    )
    maps = []
    for m in range(N_CORES):
        sl = slice(m * pl.SBB, (m + 1) * pl.SBB)
        maps.append(
            dict(
                shared,
                xm=np.ascontiguousarray(pl.xg[sl]),
                wpom=np.ascontiguousarray(pl.wpo[sl]),
                wpim=np.ascontiguousarray(pl.wpi[sl]),
                sfc=np.ascontiguousarray(pl.selfc[sl].reshape(pl.SB, P).T),
                fidx=np.ascontiguousarray(pl.fwd.idx_t[m]),
                fd=np.ascontiguousarray(pl.fwd.d_t[m]),
                ridx=np.ascontiguousarray(pl.rev.idx_t[m]),
                rd=np.ascontiguousarray(pl.rev.d_t[m]),
            )
        )
    return maps


def prepare(x, edge_index, edge_weight, Wz, bz, Wr, br, Wh, bh, Wl, bl):
    x = np.asarray(x, np.float32)
    edge_index = np.asarray(edge_index)
    edge_weight = np.asarray(edge_weight, np.float32)
    pl = host_prep(x, edge_index, edge_weight)
    OUT = np.asarray(Wz).shape[-1]
    OSZ = np.asarray(Wl).shape[-1]
    nc = build_program_queued(pl, OUT, OSZ)
    maps = _in_maps(pl, np.asarray(Wz), np.asarray(Wh), np.asarray(bz),
                    np.asarray(bh), np.asarray(Wl), np.asarray(bl))
    return nc, maps, pl


def kernel(x, edge_index, edge_weight, Wz, bz, Wr, br, Wh, bh, Wl, bl):
    nc, maps, pl = prepare(x, edge_index, edge_weight, Wz, bz, Wr, br,
                           Wh, bh, Wl, bl)

    if os.environ.get("BASS_SIM"):
        from concourse.bass_interp import MultiCoreSim

        sim = MultiCoreSim(nc, num_cores=N_CORES, trace=False)
        for i, core in enumerate(sim.cores.values()):
            for k, v in maps[i].items():
                core.tensor(k)[:] = v
        sim.simulate(check_with_hw=False)
        results = [
            {"out": np.array(core.tensor("out"))} for core in sim.cores.values()
        ]
    else:
        from concourse.bass_utils import run_bass_kernel_spmd

        res = run_bass_kernel_spmd(
            nc, maps, core_ids=list(range(N_CORES)),
            trace=bool(os.environ.get("KERNEL_TRACE")),
        )
        if res.exec_time_ns is not None:
            print(f"HW exec time: {res.exec_time_ns} ns")
        results = res.results

    full = np.concatenate([r["out"] for r in results], axis=0)  # [NS, OSZ]
    return np.ascontiguousarray(full[pl.node2g]).astype(np.float32)


# revision 54
# speedup vs baseline: 1.0327x; 1.0327x over previous
"""DCRNN cell (diffusion conv GRU step, K=3) on 8 trn2 NeuronCores.

Sharding: nodes are assigned to 8 cores x SB blocks of 128 slots by a greedy
2-D balanced bin packing (in-degree and out-degree per bin).  Each core owns
the edges whose destination falls in its node range (per direction), does
gather (indirect DMA, 4 SWDGE queues round-robin) + one-hot-selector matmul
scatter into PSUM for both diffusion hops, with one AllGather halo exchange
of the scaled hop-1 results between hops.  Gates/head are dense matmuls on
the owned slice.

Self-loop edges (row==col from the explicit loop set) are pulled out of the
edge lists and applied as local per-node terms, which drops the per-block
chunk count.  The source-node table is split at SPLIT (not NS/2) so the two
int16 index halves pack chunks tighter (5+4 instead of 6+6 per block).

Since H0 = 0 in the reference, only the first IN_CH rows of the gate weights
matter and the R gate has no effect on the output; this kernel exploits both.
"""

import os
import sys

for _p in ("/opt/pypackages", "/opt/trn_rl_repo"):
    if _p not in sys.path:
        sys.path.insert(0, _p)

from contextlib import ExitStack

import numpy as np

import concourse.bass as bass
import concourse.mybir as mybir
import concourse.tile as tile
from concourse import bacc
from concourse.bass import AP
from concourse.library_config import mlp as mlp_library
from concourse.masks import make_identity

F16 = mybir.dt.float16
F32 = mybir.dt.float32
I16 = mybir.dt.int16
I32 = mybir.dt.int32

N_CORES = 8
P = 128  # partitions / block size
WG = 4  # dst blocks per gather window


def _ceil_div(a, b):
    return -(-a // b)


# ----------------------------------------------------------------------------
# Host-side prep: permutation, edge bucketing, padded layouts (index work only)
# ----------------------------------------------------------------------------


class HostPlan:
    pass


def host_prep(x, edge_index, edge_weight):
    n, IN = x.shape
    row = edge_index[0].astype(np.int64)
    col = edge_index[1].astype(np.int64)
    w = edge_weight.astype(np.float32)
    E = row.shape[0]

    SB = _ceil_div(n, N_CORES * P)  # blocks per core
    NS = N_CORES * SB * P  # total node slots
    SBB = SB * P  # slots per core

    # deg/xtab group size: nodes per partition-row per pipeline step
    nrow = NS // P
    G = max(g for g in range(1, 29) if nrow % g == 0)
    NBG = nrow // G  # pipeline groups

    # split point of the source table (int16 index range per half)
    if os.environ.get("KERNEL_FORCE_SPLIT") and NS > G * P:
        SPLIT = (NS // (2 * G * P)) * G * P  # debug: exercise split path
    elif NS <= 32768:
        SPLIT = NS
    else:
        tgt = int(round(0.5714 * NS / (G * P))) * G * P
        SPLIT = min(32768 // (G * P) * (G * P), max(NS - 32768, tgt))
    assert SPLIT <= 32768 and NS - SPLIT <= 32768 and SPLIT % (G * P) == 0

    # --- balanced assignment of nodes to (core, block) bins ---
    din = np.bincount(col, minlength=n).astype(np.float64)
    dout = np.bincount(row, minlength=n).astype(np.float64)
    nbins = N_CORES * SB
    order = np.argsort(-(din + dout), kind="stable")
    in_load = np.zeros(nbins)
    out_load = np.zeros(nbins)
    cap = np.full(nbins, P, np.int64)
    binof = np.empty(n, np.int64)
    for nd in order:
        score = (in_load + din[nd]) ** 2 + (out_load + dout[nd]) ** 2
        score[cap == 0] = np.inf
        b = int(np.argmin(score))
        binof[nd] = b
        in_load[b] += din[nd]
        out_load[b] += dout[nd]
        cap[b] -= 1
    node2g = np.empty(n, np.int64)
    o = np.argsort(binof, kind="stable")
    rank = np.arange(n) - np.searchsorted(binof[o], binof[o])
    node2g[o] = binof[o] * P + rank

    xg = np.zeros((NS, IN), np.float32)
    xg[node2g] = x

    # --- padded per-node weight lists (for degree computation; incl. loops) ---
    def wpad(keys_g):
        o = np.argsort(keys_g, kind="stable")
        ks = keys_g[o]
        ws = w[o]
        starts = np.searchsorted(ks, np.arange(NS))
        r = np.arange(E) - starts[ks]
        cdeg = max(8, int(_ceil_div(int(r.max()) + 1, 4) * 4))
        W = np.zeros((NS, cdeg), np.float16)
        W[ks, r] = ws.astype(np.float16)
        c = np.bincount(ks, minlength=NS)
        W[c == 0, 0] = 1.0  # pad/isolated nodes: deg := 1 (never used)
        return W, cdeg

    wpo, cdeg_o = wpad(node2g[row])
    wpi, cdeg_i = wpad(node2g[col])
    CDEG = max(cdeg_o, cdeg_i)
    if cdeg_o < CDEG:
        wpo = np.pad(wpo, ((0, 0), (0, CDEG - cdeg_o)))
    if cdeg_i < CDEG:
        wpi = np.pad(wpi, ((0, 0), (0, CDEG - cdeg_i)))

    # --- per-direction edge bucketing ---
    # Pull out self-loop edges and apply them as local per-node terms instead
    # of gathered edges.  selfc[slot] counts the excluded edges per node (the
    # device multiplies the local term by it), so any multiplicity of
    # self-edges — or none — stays exact.
    selfm = row == col
    selfc_n = np.bincount(row[selfm], minlength=n).astype(np.float32)
    selfc = np.zeros(NS, np.float32)
    selfc[node2g] = selfc_n
    nonself = ~selfm
    wins = [range(s, min(s + WG, SB)) for s in range(0, SB, WG)]

    def make_dir(src_g, dst_g):
        Ed = src_g.shape[0]
        owner = dst_g // SBB
        blk = (dst_g % SBB) // P
        dslot = dst_g % P
        half = (src_g >= SPLIT).astype(np.int64)
        idxv = (src_g - half * SPLIT).astype(np.int64)
        o = np.lexsort((half, blk, owner))
        owner_s, blk_s, half_s = owner[o], blk[o], half[o]
        idx_s, dslot_s = idxv[o], dslot[o]
        # chunk capacity per (block, half): max over cores
        counts = np.zeros((N_CORES, SB, 2), np.int64)
        np.add.at(counts, (owner_s, blk_s, half_s), 1)
        C = _ceil_div(counts, P).max(axis=0)  # [SB, 2]
        # flat chunk layout: for win: for half: for blk in win
        start_chunk = np.zeros((SB, 2), np.int64)
        ct = 0
        for wi in wins:
            for h in (0, 1):
                for b in wi:
                    start_chunk[b, h] = ct
                    ct += C[b, h]
        NCH = ct
        EF = NCH * P
        # scatter edges into flat arrays
        gk = (owner_s * SB + blk_s) * 2 + half_s
        gstart = np.searchsorted(gk, np.arange(N_CORES * SB * 2))
        r = np.arange(Ed) - gstart[gk]
        posf = start_chunk[blk_s, half_s] * P + r
        idx_flat = np.zeros((N_CORES, EF), np.int16)
        d_flat = np.full((N_CORES, EF), -1.0, np.float16)
        idx_flat[owner_s, posf] = idx_s.astype(np.int16)
        d_flat[owner_s, posf] = dslot_s.astype(np.float16)
        # device layouts
        idx_t = np.ascontiguousarray(
            np.tile(idx_flat.reshape(N_CORES, EF // 16, 16).transpose(0, 2, 1), (1, 8, 1))
        )  # [N_CORES, 128, EF//16]
        d_t = np.ascontiguousarray(d_flat.reshape(N_CORES, EF // P, P).transpose(0, 2, 1))
        d = HostPlan()
        d.C = C
        d.start_chunk = start_chunk
        d.NCH = NCH
        d.EF = EF
        d.idx_t = idx_t
        d.d_t = d_t
        return d

    fwd = make_dir(node2g[row[nonself]], node2g[col[nonself]])
    rev = make_dir(node2g[col[nonself]], node2g[row[nonself]])

    pl = HostPlan()
    pl.n, pl.IN, pl.SB, pl.NS, pl.SBB, pl.CDEG = n, IN, SB, NS, SBB, CDEG
    pl.SPLIT, pl.G, pl.NBG = SPLIT, G, NBG
    pl.wins = wins
    pl.node2g = node2g
    pl.xg = xg
    pl.selfc = selfc
    pl.wpo, pl.wpi = wpo, wpi
    pl.fwd, pl.rev = fwd, rev
    return pl


# ----------------------------------------------------------------------------
# Device program
# ----------------------------------------------------------------------------


def build_program(pl, OUT, OSZ, gq_plan=None):
    """OUT: gate output channels (128); OSZ: final head size (12).

    gq_plan: optional list mapping gather emission index -> SWDGE queue.
    The Tile scheduler assigns DMASW sem lanes round-robin in *scheduled*
    order and each lane is locked to one queue, so queue numbers must equal
    scheduled_position %% 4.  build() runs twice: pass 1 with all-queue-0
    discovers the schedule, pass 2 bakes the matching queue plan.
    """
    IN, SB, NS, SBB, SPLIT, CDEG = pl.IN, pl.SB, pl.NS, pl.SBB, pl.SPLIT, pl.CDEG
    G, NBG = pl.G, pl.NBG
    NBG0 = SPLIT // (G * P)  # groups in half 0
    NBG1 = NBG - NBG0

    nc = bacc.Bacc(
        "TRN2", target_bir_lowering=False, debug=False, num_devices=N_CORES,
        enable_asserts=False, num_swdge_queues=4,
    )

    # ---- I/O ----
    xg_d = nc.dram_tensor("xg", [NS, IN], F32, kind="ExternalInput").ap()
    wpo_d = nc.dram_tensor("wpo", [NS, CDEG], F16, kind="ExternalInput").ap()
    wpi_d = nc.dram_tensor("wpi", [NS, CDEG], F16, kind="ExternalInput").ap()
    xm_d = nc.dram_tensor("xm", [SBB, IN], F32, kind="ExternalInput").ap()
    wpom_d = nc.dram_tensor("wpom", [SBB, CDEG], F16, kind="ExternalInput").ap()
    wpim_d = nc.dram_tensor("wpim", [SBB, CDEG], F16, kind="ExternalInput").ap()
    fidx_d = nc.dram_tensor("fidx", [P, pl.fwd.EF // 16], I16, kind="ExternalInput").ap()
    fd_d = nc.dram_tensor("fd", [P, pl.fwd.EF // P], F16, kind="ExternalInput").ap()
    ridx_d = nc.dram_tensor("ridx", [P, pl.rev.EF // 16], I16, kind="ExternalInput").ap()
    rd_d = nc.dram_tensor("rd", [P, pl.rev.EF // P], F16, kind="ExternalInput").ap()
    wz_d = nc.dram_tensor("wz", [2, 3, IN, OUT], F32, kind="ExternalInput").ap()
    wh_d = nc.dram_tensor("wh", [2, 3, IN, OUT], F32, kind="ExternalInput").ap()
    bz_d = nc.dram_tensor("bzc", [OUT, 1], F32, kind="ExternalInput").ap()
    bh_d = nc.dram_tensor("bhc", [OUT, 1], F32, kind="ExternalInput").ap()
    wl_d = nc.dram_tensor("wl", [OUT, OSZ], F32, kind="ExternalInput").ap()
    blr_d = nc.dram_tensor("blr", [P, OSZ], F32, kind="ExternalInput").ap()
    sfc_d = nc.dram_tensor("sfc", [P, SB], F32, kind="ExternalInput").ap()
    out_d = nc.dram_tensor("out", [SBB, OSZ], F32, kind="ExternalOutput").ap()

    xtab0 = nc.dram_tensor("xtab0", [SPLIT, 2 * IN], F16, kind="Internal").ap()
    xtab1 = (
        nc.dram_tensor("xtab1", [NS - SPLIT, 2 * IN], F16, kind="Internal").ap()
        if NBG1 > 0 else None
    )
    t1slice = nc.dram_tensor("t1slice", [SBB, 2 * IN], F16, kind="Internal").ap()
    t1tab = nc.dram_tensor(
        "t1tab", [NS, 2 * IN], F16, kind="Internal", addr_space="Shared"
    ).ap()

    with tile.TileContext(nc) as tc:
        sbuf = lambda nm, sh, dt: nc.alloc_sbuf_tensor(nm, sh, dt).ap()

        nc.gpsimd.load_library(mlp_library)

        # ---- static SBUF ----
        iota_i = sbuf("iota_i", [P, P], I32)
        iota16 = sbuf("iota16", [P, P], F16)
        ident = sbuf("ident", [P, P], F32)
        ident16 = sbuf("ident16", [P, P], F16)
        fidx_s = sbuf("fidx_s", [P, pl.fwd.EF // 16], I16)
        fd_s = sbuf("fd_s", [P, pl.fwd.EF // P], F16)
        ridx_s = sbuf("ridx_s", [P, pl.rev.EF // 16], I16)
        rd_s = sbuf("rd_s", [P, pl.rev.EF // P], F16)
        recm_o = sbuf("recm_o", [P, SB], F32)  # my recip deg, block-col packed
        recm_i = sbuf("recm_i", [P, SB], F32)
        sfc_s = sbuf("sfc_s", [P, SB], F32)  # self-edge count per slot
        X_sb = sbuf("X_sb", [P, SB * IN], F32)  # my x, block-col packed
        # local-term buffers, added into PSUM via identity "ghost" matmuls:
        # hold XR = x*recm*selfc during hop 1, then are overwritten with
        # B2h = selfc*t1loc - x/2 for hop 2 (WAR dep orders the overwrite
        # after the last hop-1 read)
        XR_o = sbuf("XR_o", [P, SB * IN], F16)
        XR_i = sbuf("XR_i", [P, SB * IN], F16)
        B2_o = XR_o
        B2_i = XR_i
        To1_sb = sbuf("To1_sb", [P, SB * IN], F16)
        Ti1_sb = sbuf("Ti1_sb", [P, SB * IN], F16)
        To2_sb = sbuf("To2_sb", [P, SB * IN], F16)
        Ti2_sb = sbuf("Ti2_sb", [P, SB * IN], F16)
        T1o_st = sbuf("T1o_st", [P, SB * IN], F16)  # staged t1 rows (per dir)
        T1i_st = sbuf("T1i_st", [P, SB * IN], F16)
        bz_s = sbuf("bz_s", [OUT, 1], F32)
        bh_s = sbuf("bh_s", [OUT, 1], F32)
        wl_s = sbuf("wl_s", [OUT, OSZ], F16)
        blr_s = sbuf("blr_s", [P, OSZ], F32)
        BZ = [sbuf(f"BZ{i}", [IN, OUT], F16) for i in range(5)]
        BH = [sbuf(f"BH{i}", [IN, OUT], F16) for i in range(5)]
        rec_all = {nm: sbuf(f"recg_{nm}", [P, G * NBG], F32) for nm in ("o", "i")}

        nc.gpsimd.iota(iota_i, [[1, P]], channel_multiplier=0)
        nc.vector.tensor_copy(iota16, iota_i)
        make_identity(nc, ident)
        nc.vector.tensor_copy(ident16, ident)
        nc.sync.dma_start(fidx_s, fidx_d)
        nc.sync.dma_start(fd_s, fd_d)
        nc.sync.dma_start(ridx_s, ridx_d)
        nc.sync.dma_start(rd_s, rd_d)
        nc.sync.dma_start(bz_s, bz_d)
        nc.sync.dma_start(bh_s, bh_d)
        nc.sync.dma_start(blr_s, blr_d)
        nc.sync.dma_start(sfc_s, sfc_d)

        # X_sb: partition p = node slot 128b+p of my range
        nc.sync.dma_start(
            AP(X_sb.tensor, 0, [[SB * IN, P], [IN, SB], [1, IN]]),
            AP(xm_d.tensor, 0, [[IN, P], [P * IN, SB], [1, IN]]),
        )

        # all pools stay open together so the prefix, hop, and gate phases
        # never serialize on SBUF space reuse (WAR on pool buffers)
        stack = ExitStack()
        dsb = stack.enter_context(tc.tile_pool(name="degs", bufs=2))
        one = stack.enter_context(tc.tile_pool(name="oneshot", bufs=1))
        gw = stack.enter_context(tc.tile_pool(name="gwin", bufs=2))
        sp = stack.enter_context(tc.tile_pool(name="strip", bufs=3))
        pp = stack.enter_context(tc.tile_pool(name="pprop", bufs=4, space="PSUM"))
        gs = stack.enter_context(tc.tile_pool(name="gts", bufs=2))
        gp = stack.enter_context(tc.tile_pool(name="gtp", bufs=2, space="PSUM"))
        gp2 = stack.enter_context(tc.tile_pool(name="gtp2", bufs=2, space="PSUM"))

        # ---- phase 1: degrees + reciprocals + xtab build, pipelined ----
        # xtab row g = [x[g]/deg_out[g] | x[g]/deg_in[g]] fp16.  Emission
        # order front-loads the critical path: xtab half 0 first (gates the
        # first gathers), then recm/XR (hop-1 posts), then half 1, then the
        # gate weights (needed last).
        def emit_group(j):
            rb4 = {}
            for nm, wsrc in (("o", wpo_d), ("i", wpi_d)):
                rec = rec_all[nm]
                wt = dsb.tile([P, G * CDEG], F16, tag=f"wt{nm}")
                nc.sync.dma_start(
                    wt[:],
                    AP(wsrc.tensor, j * G * P * CDEG,
                       [[G * CDEG, P], [1, G * CDEG]]),
                )
                dg = dsb.tile([P, G], F32, tag=f"dg{nm}")
                nc.vector.tensor_reduce(
                    dg[:],
                    wt[:].rearrange("p (j c) -> p j c", c=CDEG),
                    axis=mybir.AxisListType.X, op=mybir.AluOpType.add,
                )
                nc.vector.reciprocal(rec[:, G * j : G * j + G], dg[:])
                rb4[nm] = rec[:, G * j : G * j + G]
            xt = dsb.tile([P, G * IN], F32, tag="xin")
            nc.sync.dma_start(
                xt[:],
                AP(xg_d.tensor, j * G * P * IN, [[G * IN, P], [1, G * IN]]),
            )
            ot = dsb.tile([P, G * 2 * IN], F16, tag="xout")
            xt3 = xt[:].rearrange("p (j f) -> p j f", f=IN)
            ot3 = ot[:].rearrange("p (j f) -> p j f", f=2 * IN)
            for hx, nm in ((0, "o"), (1, "i")):
                rb = rb4[nm].unsqueeze(2).to_broadcast([P, G, IN])
                nc.vector.tensor_tensor(
                    ot3[:, :, hx * IN : (hx + 1) * IN], xt3, rb,
                    op=mybir.AluOpType.mult,
                )
            dstt, jj = (xtab0, j) if j < NBG0 else (xtab1, j - NBG0)
            nc.sync.dma_start(
                AP(dstt.tensor, jj * G * P * 2 * IN,
                   [[G * 2 * IN, P], [1, G * 2 * IN]]),
                ot[:],
            )

        for j in range(NBG0):
            emit_group(j)

        # my recips, block-column packing (one DMA + reduce per direction)
        for nm, wsrc, dst in (("o", wpom_d, recm_o), ("i", wpim_d, recm_i)):
            wt = one.tile([P, SB * CDEG], F16, tag=f"wtm{nm}")
            nc.scalar.dma_start(
                wt[:],
                AP(wsrc.tensor, 0, [[CDEG, P], [P * CDEG, SB], [1, CDEG]]),
            )
            dg = one.tile([P, SB], F32, tag=f"dgm{nm}")
            nc.vector.tensor_reduce(
                dg[:],
                wt[:].rearrange("p (j c) -> p j c", c=CDEG),
                axis=mybir.AxisListType.X, op=mybir.AluOpType.add,
            )
            nc.vector.reciprocal(dst, dg[:])

        # self-loop terms: XR = X * recm * selfc (per-direction)
        X3 = X_sb[:].rearrange("p (b f) -> p b f", f=IN)
        rs_o = sbuf("rs_o", [P, SB], F32)
        rs_i = sbuf("rs_i", [P, SB], F32)
        for recm, rs, XR in ((recm_o, rs_o, XR_o), (recm_i, rs_i, XR_i)):
            nc.vector.tensor_tensor(rs, recm, sfc_s, op=mybir.AluOpType.mult)
            nc.vector.tensor_tensor(
                XR[:].rearrange("p (b f) -> p b f", f=IN),
                X3,
                rs[:].unsqueeze(2).to_broadcast([P, SB, IN]),
                op=mybir.AluOpType.mult,
            )

        for j in range(NBG0, NBG):
            emit_group(j)

        # ---- weights prep ----
        for (src, dst) in ((wz_d, BZ), (wh_d, BH)):
            t0 = one.tile([IN, OUT], F32, tag="w0")
            t1 = one.tile([IN, OUT], F32, tag="w1")
            nc.scalar.dma_start(t0, src[0, 0])
            nc.scalar.dma_start(t1, src[1, 0])
            nc.vector.tensor_tensor(dst[0], t0, t1, op=mybir.AluOpType.add)
            for k, (di, ki) in enumerate(((0, 1), (1, 1), (0, 2), (1, 2))):
                tk = one.tile([IN, OUT], F32, tag=f"wk{k}")
                nc.scalar.dma_start(tk, src[di, ki])
                nc.vector.tensor_copy(dst[1 + k], tk)
        twl = one.tile([OUT, OSZ], F32, tag="wl")
        nc.scalar.dma_start(twl, wl_d)
        nc.vector.tensor_copy(wl_s, twl)

        # ---- gather-scatter hop machinery ----
        gq = [0]  # gather emission counter
        gather_names = []  # emission-order instruction names

        def run_hops(jobs, win_cb=None):
            """jobs: list of (dirp, idx_s, d_s, (tab_lo, tab_hi), coloff, loc,
            post).  loc[:, b*IN:] is added into the block's PSUM via an
            identity ghost matmul; post(b, psum_ap) consumes the result.
            win_cb(blocks) is emitted after each window's jobs so downstream
            per-block work interleaves with the next window's gathers."""
            if True:
                for wi in pl.wins:
                    blocks = list(wi)
                    for jid, (dirp, idx_s, d_s, tabs, coloff, loc, post) in enumerate(jobs):
                        C = dirp.C
                        nch_h = [sum(int(C[b, h]) for b in blocks) for h in (0, 1)]
                        nw = nch_h[0] + nch_h[1]
                        if nw == 0:
                            continue
                        wbuf = gw.tile([P, nw, 2 * IN], F16, tag=f"wb{jid}")
                        ct0 = int(dirp.start_chunk[blocks[0], 0])
                        for h in (0, 1):
                            ni = nch_h[h] * P
                            if ni == 0:
                                continue
                            co = 0 if h == 0 else nch_h[0]
                            qn = gq_plan[gq[0]] if gq_plan else 0
                            gi = nc.gpsimd.dma_gather(
                                wbuf[:, co : co + nch_h[h], :],
                                tabs[h],
                                idx_s[:, (ct0 + co) * 8 : (ct0 + co) * 8 + ni // 16],
                                ni, ni, 2 * IN,
                                single_packet=False,
                                queue_num=qn,
                            )
                            gather_names.append(gi.ins.name)
                            gq[0] += 1
                        # one-hot selector strips for the whole window
                        st = sp.tile([P, nw, P], F16, tag="st")
                        nc.vector.tensor_tensor(
                            st[:],
                            iota16.unsqueeze(1).to_broadcast([P, nw, P]),
                            d_s[:, ct0 : ct0 + nw].unsqueeze(2).to_broadcast([P, nw, P]),
                            op=mybir.AluOpType.is_equal,
                        )
                        for b in blocks:
                            tc_chunks = []
                            for h in (0, 1):
                                s = int(dirp.start_chunk[b, h])
                                for k in range(int(C[b, h])):
                                    tc_chunks.append(s + k)
                            ps = pp.tile([P, IN], F32, tag="ps")
                            for i, ct in enumerate(tc_chunks):
                                lw = ct - ct0  # local chunk within window buffer
                                nc.tensor.matmul(
                                    ps[:],
                                    lhsT=st[:, lw, :],
                                    rhs=wbuf[:, lw, coloff : coloff + IN],
                                    start=(i == 0),
                                    stop=False,
                                )
                            # local term: ps += I.T @ loc_b
                            nc.tensor.matmul(
                                ps[:],
                                lhsT=ident16,
                                rhs=loc[:, b * IN : (b + 1) * IN],
                                start=(len(tc_chunks) == 0),
                                stop=True,
                            )
                            post(b, ps)
                    if win_cb is not None:
                        win_cb(blocks)

        # ---- phase 3: hop 1 (+ t1 staging) ----
        # ghost matmul already added XR into ps; posts run on the Scalar engine
        def post_hop1(To_sb, recm, T1st):
            def post(b, ps):
                nc.scalar.copy(To_sb[:, b * IN : (b + 1) * IN], ps[:])
                nc.scalar.activation(
                    T1st[:, b * IN : (b + 1) * IN], ps[:],
                    mybir.ActivationFunctionType.Copy,
                    scale=recm[:, b : b + 1],
                )
            return post

        def stage_t1(blocks):
            b0, nb = blocks[0], len(blocks)
            for hx, T1st in ((0, T1o_st), (1, T1i_st)):
                nc.sync.dma_start(
                    AP(t1slice.tensor, b0 * P * 2 * IN + hx * IN,
                       [[2 * IN, P], [P * 2 * IN, nb], [1, IN]]),
                    T1st[:, b0 * IN : (b0 + nb) * IN].rearrange(
                        "p (b f) -> p b f", f=IN),
                )

        xtabs = (xtab0, xtab1 if xtab1 is not None else xtab0)
        run_hops([
            (pl.fwd, fidx_s, fd_s, xtabs, 0, XR_o, post_hop1(To1_sb, recm_o, T1o_st)),
            (pl.rev, ridx_s, rd_s, xtabs, IN, XR_i, post_hop1(Ti1_sb, recm_i, T1i_st)),
        ], win_cb=stage_t1)
        if os.environ.get("KERNEL_NO_COLL"):
            # debug: skip cross-core exchange (numerically wrong on >1 core)
            for m in range(N_CORES):
                nc.sync.dma_start(
                    t1tab[m * SBB : (m + 1) * SBB], t1slice
                )
        else:
            nc.gpsimd.collective_compute(
                "AllGather",
                mybir.AluOpType.bypass,
                replica_groups=[list(range(N_CORES))],
                ins=[t1slice],
                outs=[t1tab],
            )

        # hop2 local terms for the ghost matmul: B2h = selfc * t1loc - X/2
        # (final T2 = 2*ps then gives 2*selfc*t1loc - X + 2*chunks)
        for T1st, B2 in ((T1o_st, B2_o), (T1i_st, B2_i)):
            B23 = B2[:].rearrange("p (b f) -> p b f", f=IN)
            nc.vector.tensor_tensor(
                B23,
                T1st[:].rearrange("p (b f) -> p b f", f=IN),
                sfc_s[:].unsqueeze(2).to_broadcast([P, SB, IN]),
                op=mybir.AluOpType.mult,
            )
            nc.vector.scalar_tensor_tensor(
                B23, X3, -0.5, B23,
                op0=mybir.AluOpType.mult,
                op1=mybir.AluOpType.add,
            )

        # ---- phase 4: hop 2 ----
        def post_hop2(T2_sb):
            def post(b, ps):
                nc.scalar.activation(
                    T2_sb[:, b * IN : (b + 1) * IN], ps[:],
                    mybir.ActivationFunctionType.Copy, scale=2.0,
                )
            return post

        # ---- phase 5: gates + head (emitted per hop-2 window) ----
        GB = 2  # blocks per gate group
        comps = [X_sb, To1_sb, Ti1_sb, To2_sb, Ti2_sb]

        def emit_gates(b0, nb):
            compT = []
            for ci, csb in enumerate(comps):
                pT = gp.tile([IN, GB * P], F32, tag="pT")
                if csb.tensor.dtype == F16:
                    pT16 = pT[:].bitcast(F16)
                    for jj in range(nb):
                        nc.tensor.transpose(
                            pT16[:, jj * P : (jj + 1) * P],
                            csb[:, (b0 + jj) * IN : (b0 + jj + 1) * IN],
                            ident16,
                        )
                    psrc = pT16
                else:
                    for jj in range(nb):
                        nc.tensor.transpose(
                            pT[:, jj * P : (jj + 1) * P],
                            csb[:, (b0 + jj) * IN : (b0 + jj + 1) * IN],
                            ident,
                        )
                    psrc = pT[:]
                cT = gs.tile([IN, GB * P], F16, tag=f"cT{ci}")
                nc.scalar.copy(cT[:, : nb * P], psrc[:, : nb * P])
                compT.append(cT)
            res = {}
            for nm, BW, bias, fn in (
                ("z", BZ, bz_s, mybir.ActivationFunctionType.Sigmoid),
                ("h", BH, bh_s, mybir.ActivationFunctionType.Tanh),
            ):
                pg = gp2.tile([OUT, GB * P], F32, tag="pg")
                for ci in range(5):
                    nc.tensor.matmul(
                        pg[:, : nb * P],
                        lhsT=BW[ci],
                        rhs=compT[ci][:, : nb * P],
                        start=(ci == 0),
                        stop=(ci == 4),
                    )
                act = gs.tile([OUT, GB * P], F16, tag=f"act{nm}")
                nc.scalar.activation(act[:, : nb * P], pg[:, : nb * P], fn, bias=bias)
                res[nm] = act
            omz = gs.tile([OUT, GB * P], F16, tag="omz")
            nc.scalar.activation(
                omz[:, : nb * P], res["z"][:, : nb * P],
                mybir.ActivationFunctionType.Copy, bias=1.0, scale=-1.0,
            )
            hT = gs.tile([OUT, GB * P], F16, tag="hT")
            nc.vector.tensor_tensor(
                hT[:, : nb * P], omz[:, : nb * P], res["h"][:, : nb * P],
                op=mybir.AluOpType.mult,
            )
            hR = gs.tile([OUT, GB * P], F16, tag="hR")
            nc.scalar.activation(
                hR[:, : nb * P], hT[:, : nb * P], mybir.ActivationFunctionType.Relu,
            )
            osb = gs.tile([P, GB * OSZ], F32, tag="osb")
            for jj in range(nb):
                ph = gp2.tile([P, OSZ], F32, tag="pg")
                nc.tensor.matmul(
                    ph[:], lhsT=hR[:, jj * P : (jj + 1) * P], rhs=wl_s,
                    start=True, stop=True,
                )
                nc.vector.tensor_tensor(
                    osb[:, jj * OSZ : (jj + 1) * OSZ], ph[:], blr_s,
                    op=mybir.AluOpType.add,
                )
            nc.scalar.dma_start(
                AP(out_d.tensor, b0 * P * OSZ,
                   [[OSZ, P], [P * OSZ, nb], [1, OSZ]]),
                osb[:].rearrange("p (j s) -> p j s", s=OSZ)[:, :nb, :],
            )

        def gates_cb(blocks):
            b0, nw = blocks[0], len(blocks)
            for g0 in range(b0, b0 + nw, GB):
                emit_gates(g0, min(GB, b0 + nw - g0))

        t1tabs = (t1tab, t1tab[SPLIT:] if SPLIT < NS else t1tab)
        run_hops([
            (pl.fwd, fidx_s, fd_s, t1tabs, 0, B2_o, post_hop2(To2_sb)),
            (pl.rev, ridx_s, rd_s, t1tabs, IN, B2_i, post_hop2(Ti2_sb)),
        ], win_cb=gates_cb)

        stack.close()

    nc.compile()
    nc._gather_names = gather_names
    return nc


def _swdge_sched_order(nc):
    """Names of Pool-engine SWDGE DMA instructions in scheduled order."""
    import concourse.mybir as mb

    names = []
    for bb in nc.m.functions[0].blocks:
        for inst in bb.instructions:
            if isinstance(inst, mb.InstDMAGatherAnt):
                names.append(inst.name)
    return names


def build_program_queued(pl, OUT, OSZ):
    """Two-pass build: discover the scheduled SWDGE order, then assign
    queue = scheduled_position %% 4 so DMASW sem lanes stay single-queue."""
    nc = build_program(pl, OUT, OSZ)
    sched = _swdge_sched_order(nc)
    emit_idx = {nm: i for i, nm in enumerate(nc._gather_names)}
    if sched and len(sched) == len(emit_idx):
        plan = [0] * len(sched)
        for pos, nm in enumerate(sched):
            plan[emit_idx[nm]] = pos % 4
        nc2 = build_program(pl, OUT, OSZ, gq_plan=plan)
        # verify lane/queue consistency under the (identical) schedule
        sched2 = _swdge_sched_order(nc2)
        emit2 = {nm: i for i, nm in enumerate(nc2._gather_names)}
        lane_q = {}
        ok = len(sched2) == len(plan)
        if ok:
            for pos, nm in enumerate(sched2):
                lane, q = pos % 8, plan[emit2[nm]]
                if lane_q.setdefault(lane, q) != q:
                    ok = False
                    break
        if ok:
            return nc2
    print("kernel: SWDGE queue plan fell back to single-queue", file=sys.stderr)
    return nc  # fall back to single-queue (correct, slower)


# ----------------------------------------------------------------------------
# Entry
# ----------------------------------------------------------------------------


def _in_maps(pl, Wz, Wh, bz, bh, Wl, bl):
    IN, OUT = pl.IN, Wz.shape[-1]
    shared = dict(
        xg=pl.xg,
        wpo=pl.wpo,
        wpi=pl.wpi,
        wz=np.ascontiguousarray(Wz[:, :, :IN, :], np.float32),
        wh=np.ascontiguousarray(Wh[:, :, :IN, :], np.float32),
        bzc=np.ascontiguousarray(bz.reshape(OUT, 1), np.float32),
        bhc=np.ascontiguousarray(bh.reshape(OUT, 1), np.float32),
        wl=np.ascontiguousarray(Wl, np.float32),
        blr=np.ascontiguousarray(np.tile(bl.reshape(1, -1), (P, 1)), np.float32),
    )
    maps = []
    for m in range(N_CORES):
        sl = slice(m * pl.SBB, (m + 1) * pl.SBB)
        maps.append(
            dict(
                shared,
                xm=np.ascontiguousarray(pl.xg[sl]),
                wpom=np.ascontiguousarray(pl.wpo[sl]),
                wpim=np.ascontiguousarray(pl.wpi[sl]),
                sfc=np.ascontiguousarray(pl.selfc[sl].reshape(pl.SB, P).T),
                fidx=np.ascontiguousarray(pl.fwd.idx_t[m]),
                fd=np.ascontiguousarray(pl.fwd.d_t[m]),
                ridx=np.ascontiguousarray(pl.rev.idx_t[m]),
                rd=np.ascontiguousarray(pl.rev.d_t[m]),
            )
        )
    return maps


def prepare(x, edge_index, edge_weight, Wz, bz, Wr, br, Wh, bh, Wl, bl):
    x = np.asarray(x, np.float32)
    edge_index = np.asarray(edge_index)
    edge_weight = np.asarray(edge_weight, np.float32)
    pl = host_prep(x, edge_index, edge_weight)
    OUT = np.asarray(Wz).shape[-1]
    OSZ = np.asarray(Wl).shape[-1]
    nc = build_program_queued(pl, OUT, OSZ)
    maps = _in_maps(pl, np.asarray(Wz), np.asarray(Wh), np.asarray(bz),
                    np.asarray(bh), np.asarray(Wl), np.asarray(bl))
    return nc, maps, pl


def kernel(x, edge_index, edge_weight, Wz, bz, Wr, br, Wh, bh, Wl, bl):
    nc, maps, pl = prepare(x, edge_index, edge_weight, Wz, bz, Wr, br,
                           Wh, bh, Wl, bl)

    if os.environ.get("BASS_SIM"):
        from concourse.bass_interp import MultiCoreSim

        sim = MultiCoreSim(nc, num_cores=N_CORES, trace=False)
        for i, core in enumerate(sim.cores.values()):
            for k, v in maps[i].items():
                core.tensor(k)[:] = v
        sim.simulate(check_with_hw=False)
        results = [
            {"out": np.array(core.tensor("out"))} for core in sim.cores.values()
        ]
    else:
        from concourse.bass_utils import run_bass_kernel_spmd

        res = run_bass_kernel_spmd(
            nc, maps, core_ids=list(range(N_CORES)),
            trace=bool(os.environ.get("KERNEL_TRACE")),
        )
        if res.exec_time_ns is not None:
            print(f"HW exec time: {res.exec_time_ns} ns")
        results = res.results

    full = np.concatenate([r["out"] for r in results], axis=0)  # [NS, OSZ]
    return np.ascontiguousarray(full[pl.node2g]).astype(np.float32)


# revision 58
# speedup vs baseline: 1.1259x; 1.0902x over previous
"""DCRNN cell (diffusion conv GRU step, K=3) on 8 trn2 NeuronCores.

Sharding: nodes are assigned to 8 cores x SB blocks of 128 slots by a greedy
2-D balanced bin packing (in-degree and out-degree per bin).  Each core owns
the edges whose destination falls in its node range (per direction), does
gather (indirect DMA, 4 SWDGE queues round-robin) + one-hot-selector matmul
scatter into PSUM for both diffusion hops, with one AllGather halo exchange
of the scaled hop-1 results between hops.  Gates/head are dense matmuls on
the owned slice.

Self-loop edges (row==col from the explicit loop set) are pulled out of the
edge lists and applied as local per-node terms, which drops the per-block
chunk count.  The source-node table is split at SPLIT (not NS/2) so the two
int16 index halves pack chunks tighter (5+4 instead of 6+6 per block).

Since H0 = 0 in the reference, only the first IN_CH rows of the gate weights
matter and the R gate has no effect on the output; this kernel exploits both.
"""

import os
import sys

for _p in ("/opt/pypackages", "/opt/trn_rl_repo"):
    if _p not in sys.path:
        sys.path.insert(0, _p)

from contextlib import ExitStack

import numpy as np

import concourse.bass as bass
import concourse.mybir as mybir
import concourse.tile as tile
from concourse import bacc
from concourse.bass import AP
from concourse.library_config import mlp as mlp_library
from concourse.masks import make_identity

F16 = mybir.dt.float16
F32 = mybir.dt.float32
I16 = mybir.dt.int16
I32 = mybir.dt.int32

N_CORES = 8
P = 128  # partitions / block size
WG = 3  # dst blocks per gather window


def _ceil_div(a, b):
    return -(-a // b)


# ----------------------------------------------------------------------------
# Host-side prep: permutation, edge bucketing, padded layouts (index work only)
# ----------------------------------------------------------------------------


class HostPlan:
    pass


def host_prep(x, edge_index, edge_weight):
    n, IN = x.shape
    row = edge_index[0].astype(np.int64)
    col = edge_index[1].astype(np.int64)
    w = edge_weight.astype(np.float32)
    E = row.shape[0]

    SB = _ceil_div(n, N_CORES * P)  # blocks per core
    NS = N_CORES * SB * P  # total node slots
    SBB = SB * P  # slots per core

    # deg/xtab group size: nodes per partition-row per pipeline step
    nrow = NS // P
    G = max(g for g in range(1, 29) if nrow % g == 0)
    NBG = nrow // G  # pipeline groups

    # split point of the source table (int16 index range per half)
    if os.environ.get("KERNEL_FORCE_SPLIT") and NS > G * P:
        SPLIT = (NS // (2 * G * P)) * G * P  # debug: exercise split path
    elif NS <= 32768:
        SPLIT = NS
    else:
        tgt = int(round(0.5714 * NS / (G * P))) * G * P
        SPLIT = min(32768 // (G * P) * (G * P), max(NS - 32768, tgt))
    assert SPLIT <= 32768 and NS - SPLIT <= 32768 and SPLIT % (G * P) == 0

    # --- balanced assignment of nodes to (core, block) bins ---
    din = np.bincount(col, minlength=n).astype(np.float64)
    dout = np.bincount(row, minlength=n).astype(np.float64)
    nbins = N_CORES * SB
    order = np.argsort(-(din + dout), kind="stable")
    in_load = np.zeros(nbins)
    out_load = np.zeros(nbins)
    cap = np.full(nbins, P, np.int64)
    binof = np.empty(n, np.int64)
    for nd in order:
        score = (in_load + din[nd]) ** 2 + (out_load + dout[nd]) ** 2
        score[cap == 0] = np.inf
        b = int(np.argmin(score))
        binof[nd] = b
        in_load[b] += din[nd]
        out_load[b] += dout[nd]
        cap[b] -= 1
    node2g = np.empty(n, np.int64)
    o = np.argsort(binof, kind="stable")
    rank = np.arange(n) - np.searchsorted(binof[o], binof[o])
    node2g[o] = binof[o] * P + rank

    xg = np.zeros((NS, IN), np.float32)
    xg[node2g] = x

    # --- padded per-node weight lists (for degree computation; incl. loops) ---
    def wpad(keys_g):
        o = np.argsort(keys_g, kind="stable")
        ks = keys_g[o]
        ws = w[o]
        starts = np.searchsorted(ks, np.arange(NS))
        r = np.arange(E) - starts[ks]
        cdeg = max(8, int(_ceil_div(int(r.max()) + 1, 4) * 4))
        W = np.zeros((NS, cdeg), np.float16)
        W[ks, r] = ws.astype(np.float16)
        c = np.bincount(ks, minlength=NS)
        W[c == 0, 0] = 1.0  # pad/isolated nodes: deg := 1 (never used)
        return W, cdeg

    wpo, cdeg_o = wpad(node2g[row])
    wpi, cdeg_i = wpad(node2g[col])
    CDEG = max(cdeg_o, cdeg_i)
    if cdeg_o < CDEG:
        wpo = np.pad(wpo, ((0, 0), (0, CDEG - cdeg_o)))
    if cdeg_i < CDEG:
        wpi = np.pad(wpi, ((0, 0), (0, CDEG - cdeg_i)))

    # --- per-direction edge bucketing ---
    # Pull out self-loop edges and apply them as local per-node terms instead
    # of gathered edges.  selfc[slot] counts the excluded edges per node (the
    # device multiplies the local term by it), so any multiplicity of
    # self-edges — or none — stays exact.
    selfm = row == col
    selfc_n = np.bincount(row[selfm], minlength=n).astype(np.float32)
    selfc = np.zeros(NS, np.float32)
    selfc[node2g] = selfc_n
    nonself = ~selfm
    wins = [range(s, min(s + WG, SB)) for s in range(0, SB, WG)]

    def make_dir(src_g, dst_g):
        Ed = src_g.shape[0]
        owner = dst_g // SBB
        blk = (dst_g % SBB) // P
        dslot = dst_g % P
        half = (src_g >= SPLIT).astype(np.int64)
        idxv = (src_g - half * SPLIT).astype(np.int64)
        o = np.lexsort((half, blk, owner))
        owner_s, blk_s, half_s = owner[o], blk[o], half[o]
        idx_s, dslot_s = idxv[o], dslot[o]
        # chunk capacity per (block, half): max over cores
        counts = np.zeros((N_CORES, SB, 2), np.int64)
        np.add.at(counts, (owner_s, blk_s, half_s), 1)
        C = _ceil_div(counts, P).max(axis=0)  # [SB, 2]
        # flat chunk layout: for win: for half: for blk in win
        start_chunk = np.zeros((SB, 2), np.int64)
        ct = 0
        for wi in wins:
            for h in (0, 1):
                for b in wi:
                    start_chunk[b, h] = ct
                    ct += C[b, h]
        NCH = ct
        EF = NCH * P
        # scatter edges into flat arrays
        gk = (owner_s * SB + blk_s) * 2 + half_s
        gstart = np.searchsorted(gk, np.arange(N_CORES * SB * 2))
        r = np.arange(Ed) - gstart[gk]
        posf = start_chunk[blk_s, half_s] * P + r
        idx_flat = np.zeros((N_CORES, EF), np.int16)
        d_flat = np.full((N_CORES, EF), -1.0, np.float16)
        idx_flat[owner_s, posf] = idx_s.astype(np.int16)
        d_flat[owner_s, posf] = dslot_s.astype(np.float16)
        # device layouts
        idx_t = np.ascontiguousarray(
            np.tile(idx_flat.reshape(N_CORES, EF // 16, 16).transpose(0, 2, 1), (1, 8, 1))
        )  # [N_CORES, 128, EF//16]
        d_t = np.ascontiguousarray(d_flat.reshape(N_CORES, EF // P, P).transpose(0, 2, 1))
        d = HostPlan()
        d.C = C
        d.start_chunk = start_chunk
        d.NCH = NCH
        d.EF = EF
        d.idx_t = idx_t
        d.d_t = d_t
        return d

    fwd = make_dir(node2g[row[nonself]], node2g[col[nonself]])
    rev = make_dir(node2g[col[nonself]], node2g[row[nonself]])

    pl = HostPlan()
    pl.n, pl.IN, pl.SB, pl.NS, pl.SBB, pl.CDEG = n, IN, SB, NS, SBB, CDEG
    pl.SPLIT, pl.G, pl.NBG = SPLIT, G, NBG
    pl.wins = wins
    pl.node2g = node2g
    pl.xg = xg
    pl.selfc = selfc
    pl.wpo, pl.wpi = wpo, wpi
    pl.fwd, pl.rev = fwd, rev
    return pl


# ----------------------------------------------------------------------------
# Device program
# ----------------------------------------------------------------------------


def build_program(pl, OUT, OSZ, gq_plan=None):
    """OUT: gate output channels (128); OSZ: final head size (12).

    gq_plan: optional list mapping gather emission index -> SWDGE queue.
    The Tile scheduler assigns DMASW sem lanes round-robin in *scheduled*
    order and each lane is locked to one queue, so queue numbers must equal
    scheduled_position %% 4.  build() runs twice: pass 1 with all-queue-0
    discovers the schedule, pass 2 bakes the matching queue plan.
    """
    IN, SB, NS, SBB, SPLIT, CDEG = pl.IN, pl.SB, pl.NS, pl.SBB, pl.SPLIT, pl.CDEG
    G, NBG = pl.G, pl.NBG
    NBG0 = SPLIT // (G * P)  # groups in half 0
    NBG1 = NBG - NBG0

    nc = bacc.Bacc(
        "TRN2", target_bir_lowering=False, debug=False, num_devices=N_CORES,
        enable_asserts=False, num_swdge_queues=4,
    )

    # ---- I/O ----
    xg_d = nc.dram_tensor("xg", [NS, IN], F32, kind="ExternalInput").ap()
    wpo_d = nc.dram_tensor("wpo", [NS, CDEG], F16, kind="ExternalInput").ap()
    wpi_d = nc.dram_tensor("wpi", [NS, CDEG], F16, kind="ExternalInput").ap()
    xm_d = nc.dram_tensor("xm", [SBB, IN], F32, kind="ExternalInput").ap()
    wpom_d = nc.dram_tensor("wpom", [SBB, CDEG], F16, kind="ExternalInput").ap()
    wpim_d = nc.dram_tensor("wpim", [SBB, CDEG], F16, kind="ExternalInput").ap()
    fidx_d = nc.dram_tensor("fidx", [P, pl.fwd.EF // 16], I16, kind="ExternalInput").ap()
    fd_d = nc.dram_tensor("fd", [P, pl.fwd.EF // P], F16, kind="ExternalInput").ap()
    ridx_d = nc.dram_tensor("ridx", [P, pl.rev.EF // 16], I16, kind="ExternalInput").ap()
    rd_d = nc.dram_tensor("rd", [P, pl.rev.EF // P], F16, kind="ExternalInput").ap()
    wz_d = nc.dram_tensor("wz", [2, 3, IN, OUT], F32, kind="ExternalInput").ap()
    wh_d = nc.dram_tensor("wh", [2, 3, IN, OUT], F32, kind="ExternalInput").ap()
    bz_d = nc.dram_tensor("bzc", [OUT, 1], F32, kind="ExternalInput").ap()
    bh_d = nc.dram_tensor("bhc", [OUT, 1], F32, kind="ExternalInput").ap()
    wl_d = nc.dram_tensor("wl", [OUT, OSZ], F32, kind="ExternalInput").ap()
    blr_d = nc.dram_tensor("blr", [P, OSZ], F32, kind="ExternalInput").ap()
    sfc_d = nc.dram_tensor("sfc", [P, SB], F32, kind="ExternalInput").ap()
    out_d = nc.dram_tensor("out", [SBB, OSZ], F32, kind="ExternalOutput").ap()

    xtab0 = nc.dram_tensor("xtab0", [SPLIT, 2 * IN], F16, kind="Internal").ap()
    xtab1 = (
        nc.dram_tensor("xtab1", [NS - SPLIT, 2 * IN], F16, kind="Internal").ap()
        if NBG1 > 0 else None
    )
    t1slice = nc.dram_tensor("t1slice", [SBB, 2 * IN], F16, kind="Internal").ap()
    t1tab = nc.dram_tensor(
        "t1tab", [NS, 2 * IN], F16, kind="Internal", addr_space="Shared"
    ).ap()

    with tile.TileContext(nc) as tc:
        sbuf = lambda nm, sh, dt: nc.alloc_sbuf_tensor(nm, sh, dt).ap()

        nc.gpsimd.load_library(mlp_library)

        # ---- static SBUF ----
        iota_i = sbuf("iota_i", [P, P], I32)
        iota16 = sbuf("iota16", [P, P], F16)
        ident = sbuf("ident", [P, P], F32)
        ident16 = sbuf("ident16", [P, P], F16)
        fidx_s = sbuf("fidx_s", [P, pl.fwd.EF // 16], I16)
        fd_s = sbuf("fd_s", [P, pl.fwd.EF // P], F16)
        ridx_s = sbuf("ridx_s", [P, pl.rev.EF // 16], I16)
        rd_s = sbuf("rd_s", [P, pl.rev.EF // P], F16)
        recm_o = sbuf("recm_o", [P, SB], F32)  # my recip deg, block-col packed
        recm_i = sbuf("recm_i", [P, SB], F32)
        sfc_s = sbuf("sfc_s", [P, SB], F32)  # self-edge count per slot
        X_sb = sbuf("X_sb", [P, SB * IN], F32)  # my x, block-col packed
        # local-term buffers, added into PSUM via identity "ghost" matmuls:
        # hold XR = x*recm*selfc during hop 1, then are overwritten with
        # B2h = selfc*t1loc - x/2 for hop 2 (WAR dep orders the overwrite
        # after the last hop-1 read)
        XR_o = sbuf("XR_o", [P, SB * IN], F16)
        XR_i = sbuf("XR_i", [P, SB * IN], F16)
        B2_o = XR_o
        B2_i = XR_i
        To1_sb = sbuf("To1_sb", [P, SB * IN], F16)
        Ti1_sb = sbuf("Ti1_sb", [P, SB * IN], F16)
        To2_sb = sbuf("To2_sb", [P, SB * IN], F16)
        Ti2_sb = sbuf("Ti2_sb", [P, SB * IN], F16)
        T1o_st = sbuf("T1o_st", [P, SB * IN], F16)  # staged t1 rows (per dir)
        T1i_st = sbuf("T1i_st", [P, SB * IN], F16)
        bz_s = sbuf("bz_s", [OUT, 1], F32)
        bh_s = sbuf("bh_s", [OUT, 1], F32)
        wl_s = sbuf("wl_s", [OUT, OSZ], F16)
        blr_s = sbuf("blr_s", [P, OSZ], F32)
        BZ = [sbuf(f"BZ{i}", [IN, OUT], F16) for i in range(5)]
        BH = [sbuf(f"BH{i}", [IN, OUT], F16) for i in range(5)]
        rec_all = {nm: sbuf(f"recg_{nm}", [P, G * NBG], F32) for nm in ("o", "i")}

        nc.gpsimd.iota(iota_i, [[1, P]], channel_multiplier=0)
        nc.vector.tensor_copy(iota16, iota_i)
        make_identity(nc, ident)
        nc.vector.tensor_copy(ident16, ident)
        nc.sync.dma_start(fidx_s, fidx_d)
        nc.sync.dma_start(fd_s, fd_d)
        nc.sync.dma_start(ridx_s, ridx_d)
        nc.sync.dma_start(rd_s, rd_d)
        nc.sync.dma_start(bz_s, bz_d)
        nc.sync.dma_start(bh_s, bh_d)
        nc.sync.dma_start(blr_s, blr_d)
        nc.sync.dma_start(sfc_s, sfc_d)

        # X_sb: partition p = node slot 128b+p of my range
        nc.sync.dma_start(
            AP(X_sb.tensor, 0, [[SB * IN, P], [IN, SB], [1, IN]]),
            AP(xm_d.tensor, 0, [[IN, P], [P * IN, SB], [1, IN]]),
        )

        # all pools stay open together so the prefix, hop, and gate phases
        # never serialize on SBUF space reuse (WAR on pool buffers)
        stack = ExitStack()
        dsb = stack.enter_context(tc.tile_pool(name="degs", bufs=2))
        one = stack.enter_context(tc.tile_pool(name="oneshot", bufs=1))
        gw = stack.enter_context(tc.tile_pool(name="gwin", bufs=3))
        sp = stack.enter_context(tc.tile_pool(name="strip", bufs=3))
        pp = stack.enter_context(tc.tile_pool(name="pprop", bufs=4, space="PSUM"))
        gs = stack.enter_context(tc.tile_pool(name="gts", bufs=2))
        gp = stack.enter_context(tc.tile_pool(name="gtp", bufs=2, space="PSUM"))
        gp2 = stack.enter_context(tc.tile_pool(name="gtp2", bufs=2, space="PSUM"))

        # ---- phase 1: degrees + reciprocals + xtab build, pipelined ----
        # xtab row g = [x[g]/deg_out[g] | x[g]/deg_in[g]] fp16.  Emission
        # order front-loads the critical path: xtab half 0 first (gates the
        # first gathers), then recm/XR (hop-1 posts), then half 1, then the
        # gate weights (needed last).
        def emit_group(j):
            rb4 = {}
            for nm, wsrc in (("o", wpo_d), ("i", wpi_d)):
                rec = rec_all[nm]
                wt = dsb.tile([P, G * CDEG], F16, tag=f"wt{nm}")
                nc.sync.dma_start(
                    wt[:],
                    AP(wsrc.tensor, j * G * P * CDEG,
                       [[G * CDEG, P], [1, G * CDEG]]),
                )
                dg = dsb.tile([P, G], F32, tag=f"dg{nm}")
                nc.vector.tensor_reduce(
                    dg[:],
                    wt[:].rearrange("p (j c) -> p j c", c=CDEG),
                    axis=mybir.AxisListType.X, op=mybir.AluOpType.add,
                )
                nc.vector.reciprocal(rec[:, G * j : G * j + G], dg[:])
                rb4[nm] = rec[:, G * j : G * j + G]
            xt = dsb.tile([P, G * IN], F32, tag="xin")
            nc.sync.dma_start(
                xt[:],
                AP(xg_d.tensor, j * G * P * IN, [[G * IN, P], [1, G * IN]]),
            )
            ot = dsb.tile([P, G * 2 * IN], F16, tag="xout")
            xt3 = xt[:].rearrange("p (j f) -> p j f", f=IN)
            ot3 = ot[:].rearrange("p (j f) -> p j f", f=2 * IN)
            for hx, nm in ((0, "o"), (1, "i")):
                rb = rb4[nm].unsqueeze(2).to_broadcast([P, G, IN])
                nc.vector.tensor_tensor(
                    ot3[:, :, hx * IN : (hx + 1) * IN], xt3, rb,
                    op=mybir.AluOpType.mult,
                )
            dstt, jj = (xtab0, j) if j < NBG0 else (xtab1, j - NBG0)
            nc.sync.dma_start(
                AP(dstt.tensor, jj * G * P * 2 * IN,
                   [[G * 2 * IN, P], [1, G * 2 * IN]]),
                ot[:],
            )

        for j in range(NBG0):
            emit_group(j)

        # my recips, block-column packing (one DMA + reduce per direction)
        for nm, wsrc, dst in (("o", wpom_d, recm_o), ("i", wpim_d, recm_i)):
            wt = one.tile([P, SB * CDEG], F16, tag=f"wtm{nm}")
            nc.scalar.dma_start(
                wt[:],
                AP(wsrc.tensor, 0, [[CDEG, P], [P * CDEG, SB], [1, CDEG]]),
            )
            dg = one.tile([P, SB], F32, tag=f"dgm{nm}")
            nc.vector.tensor_reduce(
                dg[:],
                wt[:].rearrange("p (j c) -> p j c", c=CDEG),
                axis=mybir.AxisListType.X, op=mybir.AluOpType.add,
            )
            nc.vector.reciprocal(dst, dg[:])

        # self-loop terms: XR = X * recm * selfc (per-direction)
        X3 = X_sb[:].rearrange("p (b f) -> p b f", f=IN)
        rs_o = sbuf("rs_o", [P, SB], F32)
        rs_i = sbuf("rs_i", [P, SB], F32)
        for recm, rs, XR in ((recm_o, rs_o, XR_o), (recm_i, rs_i, XR_i)):
            nc.vector.tensor_tensor(rs, recm, sfc_s, op=mybir.AluOpType.mult)
            nc.vector.tensor_tensor(
                XR[:].rearrange("p (b f) -> p b f", f=IN),
                X3,
                rs[:].unsqueeze(2).to_broadcast([P, SB, IN]),
                op=mybir.AluOpType.mult,
            )

        for j in range(NBG0, NBG):
            emit_group(j)

        # ---- weights prep ----
        for (src, dst) in ((wz_d, BZ), (wh_d, BH)):
            t0 = one.tile([IN, OUT], F32, tag="w0")
            t1 = one.tile([IN, OUT], F32, tag="w1")
            nc.scalar.dma_start(t0, src[0, 0])
            nc.scalar.dma_start(t1, src[1, 0])
            nc.vector.tensor_tensor(dst[0], t0, t1, op=mybir.AluOpType.add)
            for k, (di, ki) in enumerate(((0, 1), (1, 1), (0, 2), (1, 2))):
                tk = one.tile([IN, OUT], F32, tag=f"wk{k}")
                nc.scalar.dma_start(tk, src[di, ki])
                nc.vector.tensor_copy(dst[1 + k], tk)
        twl = one.tile([OUT, OSZ], F32, tag="wl")
        nc.scalar.dma_start(twl, wl_d)
        nc.vector.tensor_copy(wl_s, twl)

        # ---- gather-scatter hop machinery ----
        gq = [0]  # gather emission counter
        gather_names = []  # emission-order instruction names

        def run_hops(jobs, win_cb=None):
            """jobs: list of (dirp, idx_s, d_s, (tab_lo, tab_hi), coloff, loc,
            post).  loc[:, b*IN:] is added into the block's PSUM via an
            identity ghost matmul; post(b, psum_ap) consumes the result.
            win_cb(blocks) is emitted after each window's jobs so downstream
            per-block work interleaves with the next window's gathers."""
            if True:
                for wi in pl.wins:
                    blocks = list(wi)
                    for jid, (dirp, idx_s, d_s, tabs, coloff, loc, post) in enumerate(jobs):
                        C = dirp.C
                        nch_h = [sum(int(C[b, h]) for b in blocks) for h in (0, 1)]
                        nw = nch_h[0] + nch_h[1]
                        if nw == 0:
                            continue
                        wbuf = gw.tile([P, nw, 2 * IN], F16, tag=f"wb{jid}")
                        ct0 = int(dirp.start_chunk[blocks[0], 0])
                        for h in (0, 1):
                            ni = nch_h[h] * P
                            if ni == 0:
                                continue
                            co = 0 if h == 0 else nch_h[0]
                            qn = gq_plan[gq[0]] if gq_plan else 0
                            gi = nc.gpsimd.dma_gather(
                                wbuf[:, co : co + nch_h[h], :],
                                tabs[h],
                                idx_s[:, (ct0 + co) * 8 : (ct0 + co) * 8 + ni // 16],
                                ni, ni, 2 * IN,
                                single_packet=False,
                                queue_num=qn,
                            )
                            gather_names.append(gi.ins.name)
                            gq[0] += 1
                        # one-hot selector strips for the whole window
                        st = sp.tile([P, nw, P], F16, tag="st")
                        nc.vector.tensor_tensor(
                            st[:],
                            iota16.unsqueeze(1).to_broadcast([P, nw, P]),
                            d_s[:, ct0 : ct0 + nw].unsqueeze(2).to_broadcast([P, nw, P]),
                            op=mybir.AluOpType.is_equal,
                        )
                        for b in blocks:
                            tc_chunks = []
                            for h in (0, 1):
                                s = int(dirp.start_chunk[b, h])
                                for k in range(int(C[b, h])):
                                    tc_chunks.append(s + k)
                            ps = pp.tile([P, IN], F32, tag="ps")
                            for i, ct in enumerate(tc_chunks):
                                lw = ct - ct0  # local chunk within window buffer
                                nc.tensor.matmul(
                                    ps[:],
                                    lhsT=st[:, lw, :],
                                    rhs=wbuf[:, lw, coloff : coloff + IN],
                                    start=(i == 0),
                                    stop=False,
                                )
                            # local term: ps += I.T @ loc_b
                            nc.tensor.matmul(
                                ps[:],
                                lhsT=ident16,
                                rhs=loc[:, b * IN : (b + 1) * IN],
                                start=(len(tc_chunks) == 0),
                                stop=True,
                            )
                            post(b, ps)
                    if win_cb is not None:
                        win_cb(blocks)

        # ---- phase 3: hop 1 (+ t1 staging) ----
        # ghost matmul already added XR into ps; posts run on the Scalar engine
        def post_hop1(To_sb, recm, T1st):
            def post(b, ps):
                nc.scalar.copy(To_sb[:, b * IN : (b + 1) * IN], ps[:])
                nc.scalar.activation(
                    T1st[:, b * IN : (b + 1) * IN], ps[:],
                    mybir.ActivationFunctionType.Copy,
                    scale=recm[:, b : b + 1],
                )
            return post

        def stage_t1(blocks):
            b0, nb = blocks[0], len(blocks)
            for hx, T1st in ((0, T1o_st), (1, T1i_st)):
                nc.sync.dma_start(
                    AP(t1slice.tensor, b0 * P * 2 * IN + hx * IN,
                       [[2 * IN, P], [P * 2 * IN, nb], [1, IN]]),
                    T1st[:, b0 * IN : (b0 + nb) * IN].rearrange(
                        "p (b f) -> p b f", f=IN),
                )

        xtabs = (xtab0, xtab1 if xtab1 is not None else xtab0)
        run_hops([
            (pl.fwd, fidx_s, fd_s, xtabs, 0, XR_o, post_hop1(To1_sb, recm_o, T1o_st)),
            (pl.rev, ridx_s, rd_s, xtabs, IN, XR_i, post_hop1(Ti1_sb, recm_i, T1i_st)),
        ], win_cb=stage_t1)
        if os.environ.get("KERNEL_NO_COLL"):
            # debug: skip cross-core exchange (numerically wrong on >1 core)
            for m in range(N_CORES):
                nc.sync.dma_start(
                    t1tab[m * SBB : (m + 1) * SBB], t1slice
                )
        else:
            nc.gpsimd.collective_compute(
                "AllGather",
                mybir.AluOpType.bypass,
                replica_groups=[list(range(N_CORES))],
                ins=[t1slice],
                outs=[t1tab],
            )

        # hop2 local terms for the ghost matmul: B2h = selfc * t1loc - X/2
        # (final T2 = 2*ps then gives 2*selfc*t1loc - X + 2*chunks)
        for T1st, B2 in ((T1o_st, B2_o), (T1i_st, B2_i)):
            B23 = B2[:].rearrange("p (b f) -> p b f", f=IN)
            nc.vector.tensor_tensor(
                B23,
                T1st[:].rearrange("p (b f) -> p b f", f=IN),
                sfc_s[:].unsqueeze(2).to_broadcast([P, SB, IN]),
                op=mybir.AluOpType.mult,
            )
            nc.vector.scalar_tensor_tensor(
                B23, X3, -0.5, B23,
                op0=mybir.AluOpType.mult,
                op1=mybir.AluOpType.add,
            )

        # ---- phase 4: hop 2 ----
        def post_hop2(T2_sb):
            def post(b, ps):
                nc.scalar.activation(
                    T2_sb[:, b * IN : (b + 1) * IN], ps[:],
                    mybir.ActivationFunctionType.Copy, scale=2.0,
                )
            return post

        # ---- phase 5: gates + head (emitted per hop-2 window) ----
        GB = 2  # blocks per gate group
        comps = [X_sb, To1_sb, Ti1_sb, To2_sb, Ti2_sb]

        def emit_gates(b0, nb):
            compT = []
            for ci, csb in enumerate(comps):
                pT = gp.tile([IN, GB * P], F32, tag="pT")
                if csb.tensor.dtype == F16:
                    pT16 = pT[:].bitcast(F16)
                    for jj in range(nb):
                        nc.tensor.transpose(
                            pT16[:, jj * P : (jj + 1) * P],
                            csb[:, (b0 + jj) * IN : (b0 + jj + 1) * IN],
                            ident16,
                        )
                    psrc = pT16
                else:
                    for jj in range(nb):
                        nc.tensor.transpose(
                            pT[:, jj * P : (jj + 1) * P],
                            csb[:, (b0 + jj) * IN : (b0 + jj + 1) * IN],
                            ident,
                        )
                    psrc = pT[:]
                cT = gs.tile([IN, GB * P], F16, tag=f"cT{ci}")
                nc.scalar.copy(cT[:, : nb * P], psrc[:, : nb * P])
                compT.append(cT)
            res = {}
            for nm, BW, bias, fn in (
                ("z", BZ, bz_s, mybir.ActivationFunctionType.Sigmoid),
                ("h", BH, bh_s, mybir.ActivationFunctionType.Tanh),
            ):
                pg = gp2.tile([OUT, GB * P], F32, tag="pg")
                for ci in range(5):
                    nc.tensor.matmul(
                        pg[:, : nb * P],
                        lhsT=BW[ci],
                        rhs=compT[ci][:, : nb * P],
                        start=(ci == 0),
                        stop=(ci == 4),
                    )
                act = gs.tile([OUT, GB * P], F16, tag=f"act{nm}")
                nc.scalar.activation(act[:, : nb * P], pg[:, : nb * P], fn, bias=bias)
                res[nm] = act
            omz = gs.tile([OUT, GB * P], F16, tag="omz")
            nc.scalar.activation(
                omz[:, : nb * P], res["z"][:, : nb * P],
                mybir.ActivationFunctionType.Copy, bias=1.0, scale=-1.0,
            )
            hT = gs.tile([OUT, GB * P], F16, tag="hT")
            nc.vector.tensor_tensor(
                hT[:, : nb * P], omz[:, : nb * P], res["h"][:, : nb * P],
                op=mybir.AluOpType.mult,
            )
            hR = gs.tile([OUT, GB * P], F16, tag="hR")
            nc.scalar.activation(
                hR[:, : nb * P], hT[:, : nb * P], mybir.ActivationFunctionType.Relu,
            )
            osb = gs.tile([P, GB * OSZ], F32, tag="osb")
            for jj in range(nb):
                ph = gp2.tile([P, OSZ], F32, tag="pg")
                nc.tensor.matmul(
                    ph[:], lhsT=hR[:, jj * P : (jj + 1) * P], rhs=wl_s,
                    start=True, stop=True,
                )
                nc.vector.tensor_tensor(
                    osb[:, jj * OSZ : (jj + 1) * OSZ], ph[:], blr_s,
                    op=mybir.AluOpType.add,
                )
            nc.scalar.dma_start(
                AP(out_d.tensor, b0 * P * OSZ,
                   [[OSZ, P], [P * OSZ, nb], [1, OSZ]]),
                osb[:].rearrange("p (j s) -> p j s", s=OSZ)[:, :nb, :],
            )

        def gates_cb(blocks):
            b0, nw = blocks[0], len(blocks)
            for g0 in range(b0, b0 + nw, GB):
                emit_gates(g0, min(GB, b0 + nw - g0))

        t1tabs = (t1tab, t1tab[SPLIT:] if SPLIT < NS else t1tab)
        run_hops([
            (pl.fwd, fidx_s, fd_s, t1tabs, 0, B2_o, post_hop2(To2_sb)),
            (pl.rev, ridx_s, rd_s, t1tabs, IN, B2_i, post_hop2(Ti2_sb)),
        ], win_cb=gates_cb)

        stack.close()

    nc.compile()
    nc._gather_names = gather_names
    return nc


def _swdge_sched_order(nc):
    """Names of Pool-engine SWDGE DMA instructions in scheduled order."""
    import concourse.mybir as mb

    names = []
    for bb in nc.m.functions[0].blocks:
        for inst in bb.instructions:
            if isinstance(inst, mb.InstDMAGatherAnt):
                names.append(inst.name)
    return names


def build_program_queued(pl, OUT, OSZ):
    """Two-pass build: discover the scheduled SWDGE order, then assign
    queue = scheduled_position %% 4 so DMASW sem lanes stay single-queue."""
    nc = build_program(pl, OUT, OSZ)
    sched = _swdge_sched_order(nc)
    emit_idx = {nm: i for i, nm in enumerate(nc._gather_names)}
    if sched and len(sched) == len(emit_idx):
        plan = [0] * len(sched)
        for pos, nm in enumerate(sched):
            plan[emit_idx[nm]] = pos % 4
        nc2 = build_program(pl, OUT, OSZ, gq_plan=plan)
        # verify lane/queue consistency under the (identical) schedule
        sched2 = _swdge_sched_order(nc2)
        emit2 = {nm: i for i, nm in enumerate(nc2._gather_names)}
        lane_q = {}
        ok = len(sched2) == len(plan)
        if ok:
            for pos, nm in enumerate(sched2):
                lane, q = pos % 8, plan[emit2[nm]]
                if lane_q.setdefault(lane, q) != q:
                    ok = False
                    break
        if ok:
            return nc2
    print("kernel: SWDGE queue plan fell back to single-queue", file=sys.stderr)
    return nc  # fall back to single-queue (correct, slower)


# ----------------------------------------------------------------------------
# Entry
# ----------------------------------------------------------------------------


def _in_maps(pl, Wz, Wh, bz, bh, Wl, bl):
    IN, OUT = pl.IN, Wz.shape[-1]
    shared = dict(
        xg=pl.xg,
        wpo=pl.wpo,
        wpi=pl.wpi,
        wz=np.ascontiguousarray(Wz[:, :, :IN, :], np.float32),
        wh=np.ascontiguousarray(Wh[:, :, :IN, :], np.float32),
        bzc=np.ascontiguousarray(bz.reshape(OUT, 1), np.float32),
        bhc=np.ascontiguousarray(bh.reshape(OUT, 1), np.float32),
        wl=np.ascontiguousarray(Wl, np.float32),
        blr=np.ascontiguousarray(np.tile(bl.reshape(1, -1), (P, 1)), np.float32),
    )
    maps = []
    for m in range(N_CORES):
        sl = slice(m * pl.SBB, (m + 1) * pl.SBB)
        maps.append(
            dict(
                shared,
                xm=np.ascontiguousarray(pl.xg[sl]),
                wpom=np.ascontiguousarray(pl.wpo[sl]),
                wpim=np.ascontiguousarray(pl.wpi[sl]),
                sfc=np.ascontiguousarray(pl.selfc[sl].reshape(pl.SB, P).T),
                fidx=np.ascontiguousarray(pl.fwd.idx_t[m]),
                fd=np.ascontiguousarray(pl.fwd.d_t[m]),
                ridx=np.ascontiguousarray(pl.rev.idx_t[m]),
                rd=np.ascontiguousarray(pl.rev.d_t[m]),
            )
        )
    return maps


def prepare(x, edge_index, edge_weight, Wz, bz, Wr, br, Wh, bh, Wl, bl):
    x = np.asarray(x, np.float32)
    edge_index = np.asarray(edge_index)
    edge_weight = np.asarray(edge_weight, np.float32)
    pl = host_prep(x, edge_index, edge_weight)
    OUT = np.asarray(Wz).shape[-1]
    OSZ = np.asarray(Wl).shape[-1]
    nc = build_program_queued(pl, OUT, OSZ)
    maps = _in_maps(pl, np.asarray(Wz), np.asarray(Wh), np.asarray(bz),
                    np.asarray(bh), np.asarray(Wl), np.asarray(bl))
    return nc, maps, pl


def kernel(x, edge_index, edge_weight, Wz, bz, Wr, br, Wh, bh, Wl, bl):
    nc, maps, pl = prepare(x, edge_index, edge_weight, Wz, bz, Wr, br,
                           Wh, bh, Wl, bl)

    if os.environ.get("BASS_SIM"):
        from concourse.bass_interp import MultiCoreSim

        sim = MultiCoreSim(nc, num_cores=N_CORES, trace=False)
        for i, core in enumerate(sim.cores.values()):
            for k, v in maps[i].items():
                core.tensor(k)[:] = v
        sim.simulate(check_with_hw=False)
        results = [
            {"out": np.array(core.tensor("out"))} for core in sim.cores.values()
        ]
    else:
        from concourse.bass_utils import run_bass_kernel_spmd

        res = run_bass_kernel_spmd(
            nc, maps, core_ids=list(range(N_CORES)),
            trace=bool(os.environ.get("KERNEL_TRACE")),
        )
        if res.exec_time_ns is not None:
            print(f"HW exec time: {res.exec_time_ns} ns")
        results = res.results

    full = np.concatenate([r["out"] for r in results], axis=0)  # [NS, OSZ]
    return np.ascontiguousarray(full[pl.node2g]).astype(np.float32)
